# revision 1
# baseline (speedup 1.0000x reference)
"""Causal self-attention (B=4, T=2048, DIM=1024, H=8, D=128) on 8 trn2 cores.

Sharding: core i handles batch b = i//2, head-group g = i%2 (4 heads each).
Per-core: fused QKV (fp32r matmuls), per-head RMSnorm + RoPE, causal
attention in scores^T layout (softmax over the partition axis: denominator
via M=1 PE matmuls), lambda-mix of V with ve (lambdas folded host-side),
c_proj partial. Host sums the two head-group partials per batch.

All big matmuls run as float32r (TF32) at 1 cycle/row; probs/V/c_proj use
bf16 operands (fp32 PSUM accumulate).
"""
import sys

sys.path.insert(0, "/opt/trn_rl_repo")

from contextlib import ExitStack

import numpy as np
import ml_dtypes

import concourse.bass as bass  # noqa: F401
import concourse.mybir as mybir
import concourse.tile as tile
from concourse import bacc
from concourse.bass_utils import run_bass_kernel_spmd

B, T, DIM, H, D = 4, 2048, 1024, 8, 128
HG = 2              # head-groups (tensor-parallel factor per batch)
HPG = H // HG       # heads per core
CS = 512            # t-chunk size (PSUM fp32 bank = 512 cols)
NCH = T // CS       # 4 chunks
KT = T // 128       # 16 tk tiles
KD = DIM // 128     # 8 contraction tiles
FQK = HPG * 2 * 128  # 1024 qk feature cols per core
FV = HPG * 128       # 512 v cols per core
F32 = mybir.dt.float32
R32 = mybir.dt.float32r
BF16 = mybir.dt.bfloat16
EPS = float(np.finfo(np.float32).eps)
SCALE = float(D ** -0.5)
MUL = mybir.AluOpType.mult
ADD = mybir.AluOpType.add
SUB = mybir.AluOpType.subtract

_cache = {}


def _rope_tables():
    freq = (1.0 / 1024.0) ** np.linspace(0.0, 1.0, D // 4, dtype=np.float64)
    freq = np.concatenate([freq, np.zeros(D // 4)])
    theta = np.arange(T, dtype=np.float64)[:, None] * freq[None, :]  # [T, 64]
    cos = np.cos(theta).astype(np.float32).T.copy()  # [64, T]
    sin = np.sin(theta).astype(np.float32).T.copy()
    return cos, sin


def _masks():
    # mask_r[i, j] = 1 if j - i >= 128*r  (tk tile at offset r*128 inside a
    # 512-wide tq chunk); concatenated along free dim -> [128, 4*512]
    i = np.arange(128)[:, None]
    j = np.arange(CS)[None, :]
    tiles = [(j - i >= 128 * r).astype(np.float32) for r in range(4)]
    return np.concatenate(tiles, axis=1).astype(ml_dtypes.bfloat16)


def _phase_a(nc, tc, ctx, dram, P):
    """QKV projection, v-mix, RMSnorm stats, RoPE, normalize into qkT."""
    wqk_pool = ctx.enter_context(tc.tile_pool(name="w", bufs=KD))
    wv_pool = ctx.enter_context(tc.tile_pool(name="wvp", bufs=KD))
    xt_pool = ctx.enter_context(tc.tile_pool(name="xt", bufs=10))
    ve_pool = ctx.enter_context(tc.tile_pool(name="vep", bufs=2))
    raw_pool = ctx.enter_context(tc.tile_pool(name="raw", bufs=3))
    rt_pool = ctx.enter_context(tc.tile_pool(name="rtmp", bufs=4))
    rop_pool = ctx.enter_context(tc.tile_pool(name="rop", bufs=10))
    tab_pool = ctx.enter_context(tc.tile_pool(name="tab", bufs=2))
    ms_pool = ctx.enter_context(tc.tile_pool(name="ms", bufs=2))
    pa_pool = ctx.enter_context(tc.tile_pool(name="pa", bufs=4, space="PSUM"))
    pss_pool = ctx.enter_context(tc.tile_pool(name="pss", bufs=2, space="PSUM"))
    pbc_pool = ctx.enter_context(tc.tile_pool(name="pbc", bufs=2, space="PSUM"))

    w_qk = [wqk_pool.tile([128, FQK], R32, tag="wqk", name=f"wqk{i}") for i in range(KD)]
    w_v = [wv_pool.tile([128, FV], R32, tag="wv", name=f"wv{i}") for i in range(KD)]
    for kd in range(KD):
        ksl = slice(kd * 128, (kd + 1) * 128)
        nc.sync.dma_start(w_qk[kd][:], dram["wqk"].ap()[ksl, :].bitcast(R32))
        nc.sync.dma_start(w_v[kd][:], dram["wv"].ap()[ksl, :].bitcast(R32))

    for c in range(NCH):
        csl = slice(c * CS, (c + 1) * CS)
        xts = []
        for kd in range(KD):
            xt_t = xt_pool.tile([128, CS], R32, tag="xt", name=f"xt{c}_{kd}")
            nc.sync.dma_start(
                xt_t[:], dram["xt"].ap()[kd * 128:(kd + 1) * 128, csl].bitcast(R32)
            )
            xts.append(xt_t)

        # tables duplicated across both partition halves (DVE requires equal
        # base partitions when both tensor_tensor inputs are in SBUF)
        cos_t = tab_pool.tile([128, CS], F32, tag="cos")
        sin_t = tab_pool.tile([128, CS], F32, tag="sin")
        nc.sync.dma_start(cos_t[0:64, :], dram["cos"].ap()[:, csl])
        nc.sync.dma_start(cos_t[64:128, :], dram["cos"].ap()[:, csl])
        nc.sync.dma_start(sin_t[0:64, :], dram["sin"].ap()[:, csl])
        nc.sync.dma_start(sin_t[64:128, :], dram["sin"].ap()[:, csl])

        # v for this chunk's 4 token sub-tiles
        for sub in range(4):
            ti = c * 4 + sub
            pv = pa_pool.tile([128, FV], F32, tag="pa")
            for kd in range(KD):
                nc.tensor.matmul(
                    pv[:], xts[kd][:, sub * 128:(sub + 1) * 128], w_v[kd][:],
                    start=(kd == 0), stop=(kd == KD - 1),
                )
            ve_t = ve_pool.tile([128, FV], F32, tag="ve")
            nc.sync.dma_start(ve_t[:], dram["ve"].ap()[ti * 128:(ti + 1) * 128, :])
            nc.vector.tensor_tensor(P["v_bf"][ti][:], pv[:], ve_t[:], ADD)

        # q/k per head: project, sumsq, rope
        rops = []
        rstds = []
        for h in range(HPG):
            for qi in range(2):
                f0 = h * 256 + qi * 128
                pqk = pa_pool.tile([128, CS], F32, tag="pa")
                for kd in range(KD):
                    nc.tensor.matmul(
                        pqk[:], w_qk[kd][:, f0:f0 + 128], xts[kd][:],
                        start=(kd == 0), stop=(kd == KD - 1),
                    )
                raw = raw_pool.tile([128, CS], F32, tag="raw")
                nc.scalar.copy(raw[:], pqk[:])
                # mean of squares over the 128 head dims (partition axis):
                # Square(raw/sqrt(128)) summed by a ones matmul = mean
                sq = raw_pool.tile([128, CS], R32, tag="sq")
                nc.scalar.activation(
                    sq[:], raw[:], mybir.ActivationFunctionType.Square, scale=SCALE
                )
                ssps = pss_pool.tile([1, CS], F32, tag="ss")
                nc.tensor.matmul(ssps[:], P["ones_r"][:], sq[:], start=True, stop=True)
                ms_r = ms_pool.tile([1, CS], F32, tag="ms", bufs=4)
                nc.vector.tensor_scalar_add(ms_r[:], ssps[:], EPS)
                inv_r = ms_pool.tile([1, CS], F32, tag="inv", bufs=4)
                nc.vector.reciprocal(inv_r[:], ms_r[:])
                rstd = ms_pool.tile([1, CS], R32, tag="rstd", bufs=8,
                                    name=f"rstd{c}_{2 * h + qi}")
                nc.scalar.sqrt(rstd[:], inv_r[:])
                rstds.append(rstd)
                # rope: rows 0:64 = x1*c + x2*s ; rows 64:128 = x2*c - x1*s
                t_c1 = rt_pool.tile([64, CS], F32, tag="rt")
                t_s2 = rt_pool.tile([64, CS], F32, tag="rt")
                t_c2 = rt_pool.tile([64, CS], F32, tag="rt")
                t_s1 = rt_pool.tile([64, CS], F32, tag="rt")
                nc.vector.tensor_tensor(t_c1[:], raw[0:64, :], cos_t[0:64, :], MUL)
                nc.vector.tensor_tensor(t_s2[:], raw[64:128, :], sin_t[64:128, :], MUL)
                nc.vector.tensor_tensor(t_c2[:], raw[64:128, :], cos_t[64:128, :], MUL)
                nc.vector.tensor_tensor(t_s1[:], raw[0:64, :], sin_t[0:64, :], MUL)
                rop = rop_pool.tile([128, CS], F32, tag="rop")
                nc.vector.tensor_tensor(rop[0:64, :], t_c1[:], t_s2[:], ADD)
                nc.vector.tensor_tensor(rop[64:128, :], t_c2[:], t_s1[:], SUB)
                rops.append(rop)

        # normalize rope outputs into qkT
        for row in range(8):
            pbc = pbc_pool.tile([128, CS], F32, tag="bc")
            nc.tensor.matmul(
                pbc[:], P["ones1_r"][:], rstds[row][:], start=True, stop=True
            )
            nc.vector.tensor_tensor(P["qkT"][row][:, csl], rops[row][:], pbc[:], MUL)


def _phase_b(nc, tc, ctx, P):
    """Causal attention per head, scores^T layout."""
    ex_pool = ctx.enter_context(tc.tile_pool(name="exp", bufs=KT))
    sm_pool = ctx.enter_context(tc.tile_pool(name="sm", bufs=3))
    rb_pool = ctx.enter_context(tc.tile_pool(name="rb", bufs=2))
    pb_pool = ctx.enter_context(tc.tile_pool(name="pb", bufs=3, space="PSUM"))
    py_pool = ctx.enter_context(tc.tile_pool(name="py", bufs=2, space="PSUM"))
    pd_pool = ctx.enter_context(tc.tile_pool(name="pd", bufs=2, space="PSUM"))
    pn_pool = ctx.enter_context(tc.tile_pool(name="pn", bufs=1, space="PSUM"))

    for h in range(HPG):
        qh, kh = P["qkT"][2 * h], P["qkT"][2 * h + 1]
        for c in range(NCH):
            csl = slice(c * CS, (c + 1) * CS)
            nkt = 4 * (c + 1)
            exs = []
            for kt in range(nkt):
                ps = pb_pool.tile([128, CS], F32, tag="s")
                nc.tensor.matmul(
                    ps[:], kh[:, kt * 128:(kt + 1) * 128], qh[:, csl],
                    start=True, stop=True,
                )
                ex = ex_pool.tile([128, CS], BF16, tag="ex")
                nc.scalar.activation(
                    ex[:], ps[:], mybir.ActivationFunctionType.Exp, scale=SCALE
                )
                r = kt - 4 * c
                if r >= 0:
                    nc.vector.tensor_tensor(
                        ex[:], ex[:], P["mask_t"][:, r * CS:(r + 1) * CS], MUL
                    )
                exs.append(ex)
            yac = py_pool.tile([128, CS], F32, tag="y")
            den = pd_pool.tile([1, CS], F32, tag="d")
            for kt in range(nkt):
                nc.tensor.matmul(
                    yac[:], P["v_bf"][kt][:, h * 128:(h + 1) * 128], exs[kt][:],
                    start=(kt == 0), stop=(kt == nkt - 1),
                )
            for kt in range(nkt):
                nc.tensor.matmul(
                    den[:], P["ones_b"][:], exs[kt][:],
                    start=(kt == 0), stop=(kt == nkt - 1),
                )
            rcp = sm_pool.tile([1, CS], R32, tag="rcp")
            nc.vector.reciprocal(rcp[:], den[:])
            pnb = pn_pool.tile([128, CS], F32, tag="nb")
            nc.tensor.matmul(pnb[:], P["ones1_r"][:], rcp[:], start=True, stop=True)
            rbc = rb_pool.tile([128, CS], F32, tag="rb")
            nc.scalar.copy(rbc[:], pnb[:])
            nc.vector.tensor_tensor(P["yT"][h][:, csl], yac[:], rbc[:], MUL)


def _phase_c(nc, tc, ctx, dram, P):
    """c_proj partial: oT[m, t] = sum_j cwT[j, m] * yT[j, t]."""
    cw_pool = ctx.enter_context(tc.tile_pool(name="cwp", bufs=HPG))
    os_pool = ctx.enter_context(tc.tile_pool(name="os", bufs=4))
    pc_pool = ctx.enter_context(tc.tile_pool(name="pc", bufs=4, space="PSUM"))

    cwt = [cw_pool.tile([128, DIM], BF16, tag="cw", name=f"cw{i}") for i in range(HPG)]
    for j in range(HPG):
        nc.sync.dma_start(cwt[j][:], dram["cw"].ap()[j * 128:(j + 1) * 128, :])
    for m in range(KD):
        msl = slice(m * 128, (m + 1) * 128)
        for c in range(NCH):
            csl = slice(c * CS, (c + 1) * CS)
            po = pc_pool.tile([128, CS], F32, tag="pc")
            for j in range(HPG):
                nc.tensor.matmul(
                    po[:], cwt[j][:, msl], P["yT"][j][:, csl],
                    start=(j == 0), stop=(j == HPG - 1),
                )
            so = os_pool.tile([128, CS], F32, tag="os")
            nc.scalar.copy(so[:], po[:])
            nc.sync.dma_start(dram["ot"].ap()[msl, csl], so[:])


def _build_program():
    nc = bacc.Bacc("TRN2", target_bir_lowering=False, debug=False, num_devices=B * HG)

    dram = {
        "xt": nc.dram_tensor("xt", [DIM, T], F32, kind="ExternalInput"),
        "wqk": nc.dram_tensor("wqk", [DIM, FQK], F32, kind="ExternalInput"),
        "wv": nc.dram_tensor("wv", [DIM, FV], F32, kind="ExternalInput"),
        "ve": nc.dram_tensor("ve", [T, FV], F32, kind="ExternalInput"),
        "cw": nc.dram_tensor("cw", [FV, DIM], BF16, kind="ExternalInput"),
        "cos": nc.dram_tensor("cos", [64, T], F32, kind="ExternalInput"),
        "sin": nc.dram_tensor("sin", [64, T], F32, kind="ExternalInput"),
        "mask": nc.dram_tensor("mask", [128, 4 * CS], BF16, kind="ExternalInput"),
        "ones": nc.dram_tensor("ones", [128, 1], F32, kind="ExternalInput"),
        "ones1": nc.dram_tensor("ones1", [1, 128], F32, kind="ExternalInput"),
        "onesbf": nc.dram_tensor("onesbf", [128, 1], BF16, kind="ExternalInput"),
        "ot": nc.dram_tensor("ot", [DIM, T], F32, kind="ExternalOutput"),
    }

    with ExitStack() as top:
        top.enter_context(nc.allow_low_precision(reason="bf16 probs/V/c_proj by design"))
        tc = top.enter_context(tile.TileContext(nc))
        qk_pool = top.enter_context(tc.tile_pool(name="qk", bufs=2 * HPG))
        v_pool = top.enter_context(tc.tile_pool(name="vbf", bufs=KT))
        c_pool = top.enter_context(tc.tile_pool(name="const", bufs=1))

        P = {
            "qkT": [qk_pool.tile([128, T], BF16, tag="qk", name=f"qkT{i}")
                    for i in range(2 * HPG)],
            "v_bf": [v_pool.tile([128, FV], BF16, tag="v", name=f"vbf{i}")
                     for i in range(KT)],
            "ones_r": c_pool.tile([128, 1], R32, tag="ones", name="ones_r"),
            "ones1_r": c_pool.tile([1, 128], R32, tag="ones1", name="ones1_r"),
            "ones_b": c_pool.tile([128, 1], BF16, tag="onesbf", name="ones_b"),
        }
        nc.sync.dma_start(P["ones_r"][:], dram["ones"].ap().bitcast(R32))
        nc.sync.dma_start(P["ones1_r"][:], dram["ones1"].ap().bitcast(R32))
        nc.sync.dma_start(P["ones_b"][:], dram["onesbf"].ap())

        with ExitStack() as ctx_a:
            _phase_a(nc, tc, ctx_a, dram, P)
        with ExitStack() as ctx_bc:
            y_pool = ctx_bc.enter_context(tc.tile_pool(name="yt", bufs=HPG))
            m_pool = ctx_bc.enter_context(tc.tile_pool(name="maskp", bufs=1))
            P["yT"] = [y_pool.tile([128, T], BF16, tag="y", name=f"yT{i}")
                       for i in range(HPG)]
            P["mask_t"] = m_pool.tile([128, 4 * CS], BF16, tag="mask", name="mask_t")
            nc.sync.dma_start(P["mask_t"][:], dram["mask"].ap())
            with ExitStack() as ctx_b:
                _phase_b(nc, tc, ctx_b, P)
            with ExitStack() as ctx_c:
                _phase_c(nc, tc, ctx_c, dram, P)

    nc.compile()
    return nc


def _prep_inputs(x, ve, qkv_w, lambdas, c_proj_w):
    cos, sin = _rope_tables()
    mask = _masks()
    ones = np.ones((128, 1), np.float32)
    ones1 = np.ones((1, 128), np.float32)
    onesbf = np.ones((128, 1), ml_dtypes.bfloat16)
    qw, kw, vw = qkv_w[0], qkv_w[1], qkv_w[2]

    in_maps = []
    for core in range(B * HG):
        b, g = core // HG, core % HG
        heads = range(g * HPG, (g + 1) * HPG)
        rows = np.concatenate(
            [np.concatenate([qw[h * D:(h + 1) * D], kw[h * D:(h + 1) * D]])
             for h in heads]
        )                                    # [1024, DIM]
        vcols = slice(g * HPG * D, (g + 1) * HPG * D)
        in_maps.append({
            "xt": np.ascontiguousarray(x[b].T),
            "wqk": np.ascontiguousarray(rows.T),
            "wv": np.ascontiguousarray((lambdas[0] * vw[vcols]).T),
            "ve": np.ascontiguousarray(lambdas[1] * ve[b][:, vcols]),
            "cw": np.ascontiguousarray(c_proj_w[:, vcols].T).astype(ml_dtypes.bfloat16),
            "cos": cos,
            "sin": sin,
            "mask": mask,
            "ones": ones,
            "ones1": ones1,
            "onesbf": onesbf,
        })
    return in_maps


def kernel(x, ve, qkv_w, lambdas, c_proj_w):
    x = np.asarray(x, np.float32)
    ve = np.asarray(ve, np.float32)
    qkv_w = np.asarray(qkv_w, np.float32).reshape(3, H * D, DIM)
    lambdas = np.asarray(lambdas, np.float32)
    c_proj_w = np.asarray(c_proj_w, np.float32)

    if "nc" not in _cache:
        _cache["nc"] = _build_program()
    nc = _cache["nc"]

    in_maps = _prep_inputs(x, ve, qkv_w, lambdas, c_proj_w)
    res = run_bass_kernel_spmd(nc, in_maps, list(range(B * HG))).results

    out = np.empty((B, T, DIM), np.float32)
    for b in range(B):
        acc = res[HG * b]["ot"].astype(np.float32)
        for g in range(1, HG):
            acc = acc + res[HG * b + g]["ot"]
        out[b] = acc.T
    return out



# revision 10
# speedup vs baseline: 3.5988x; 3.5988x over previous
"""Causal self-attention (B=4, T=2048, DIM=1024, H=8, D=128) on 8 trn2 cores.

The axon tunnel moves ~45-50 MB/s with a per-buffer fixed cost, so wall-clock
is dominated by host<->device bytes and transfer count. This version:
  - ships ONE packed fp16 tensor per core (~5.3MB) instead of 8 tensors
    (inputs randn-scale, weights ~+-0.03: fp16-safe; fp8 was measured to
    breach the 2e-2 error budget);
  - x ships as per-core *halves* along DIM (no duplication across the two
    head-group cores of a batch); an on-device pair AllGather reconstructs it;
  - qkv/c_proj weights ship as per-core *quarters* along DIM; an on-device
    AllGather over the 4 batch-cores of each head-group reconstructs them;
  - rope tables (32 non-trivial freq rows) ship 1/8th per core, deduped by an
    8-core AllGather; causal mask and ones constants are generated on device;
  - the c_proj partial is pair-ReduceScattered on device, so each core ships
    back only its unique token-half of the final output in fp16.
Total ~42MB in + 16MB out vs 172MB + 64MB for the f32 no-collective version.

Core i handles batch b = i//2, head-group g = i%2 (4 heads each).
Compute per core: x^T via DMA-transpose, fused QKV (fp16 matmuls, fp32 PSUM),
per-head RMSnorm + RoPE, causal attention in scores^T layout (softmax
denominator via M=1 PE matmuls, mask via affine_select), lambda-mix of V with
ve (lambda0 folded into wv host-side, lambda1 applied on device), c_proj into
[token, dim] layout, pair ReduceScatter over token halves.
"""
import sys

sys.path.insert(0, "/opt/trn_rl_repo")

from contextlib import ExitStack

import numpy as np

import concourse.bass as bass  # noqa: F401
import concourse.mybir as mybir
import concourse.tile as tile
from concourse import bacc
from concourse.bass_utils import run_bass_kernel_spmd

B, T, DIM, H, D = 4, 2048, 1024, 8, 128
HG = 2              # head-groups (tensor-parallel factor per batch)
HPG = H // HG       # heads per core
CS = 512            # t-chunk size (PSUM fp32 bank = 512 cols)
NCH = T // CS       # 4 chunks
KT = T // 128       # 16 tk tiles
KD = DIM // 128     # 8 contraction tiles
FQK = HPG * 2 * 128  # 1024 qk feature cols per core
FV = HPG * 128       # 512 v cols per core
XH = DIM // 2        # per-core x half width
F32 = mybir.dt.float32
R32 = mybir.dt.float32r
F16 = mybir.dt.float16
BF16 = mybir.dt.bfloat16
EPS = float(np.finfo(np.float32).eps)
SCALE = float(D ** -0.5)
MUL = mybir.AluOpType.mult
ADD = mybir.AluOpType.add
SUB = mybir.AluOpType.subtract

PAIRS = [[2 * b, 2 * b + 1] for b in range(B)]          # x gather / out scatter
QUADS = [[g, 2 + g, 4 + g, 6 + g] for g in range(HG)]   # weight gather
ALL8 = [list(range(B * HG))]                             # rope-table gather

# packed input layout: row offsets in a [PK_ROWS, 1024] fp16 tensor
X_OFF = 0                      # x half   [2048, 512]  -> 1024 rows
WQK_OFF = X_OFF + T * XH // 1024          # wqk quarter [256, 1024] -> 256 rows
WV_OFF = WQK_OFF + 256                    # wv quarter [256, 512] -> 128 rows
CW_OFF = WV_OFF + 128                     # cw quarter [128, 1024] -> 128 rows
VE_OFF = CW_OFF + 128                     # ve half  [2048, 512]  -> 1024 rows
CSN_OFF = VE_OFF + T * FV // 1024         # cos|sin slice [8, 2048] -> 16 rows
LAM_OFF = CSN_OFF + 16                    # lambda1 fp16 at [LAM_OFF, 0]
PK_ROWS = LAM_OFF + 1

_cache = {}


def _rope_tables():
    freq = (1.0 / 1024.0) ** np.linspace(0.0, 1.0, D // 4, dtype=np.float64)
    theta = np.arange(T, dtype=np.float64)[:, None] * freq[None, :]  # [T, 32]
    cos = np.cos(theta).astype(np.float16).T.copy()  # [32, T]
    sin = np.sin(theta).astype(np.float16).T.copy()
    return np.concatenate([cos, sin])                # [64, T]


def _phase_a(nc, tc, ctx, dram, P, xg, wqkg, wvg):
    """x^T via DMA-transpose, QKV projection, v-mix, RMSnorm stats, RoPE."""
    xt_pool = ctx.enter_context(tc.tile_pool(name="xt", bufs=KD))
    wqk_pool = ctx.enter_context(tc.tile_pool(name="w", bufs=KD))
    wv_pool = ctx.enter_context(tc.tile_pool(name="wvp", bufs=KD))
    ve_pool = ctx.enter_context(tc.tile_pool(name="vep", bufs=2))
    raw_pool = ctx.enter_context(tc.tile_pool(name="raw", bufs=6))
    rt_pool = ctx.enter_context(tc.tile_pool(name="rtmp", bufs=4))
    rop_pool = ctx.enter_context(tc.tile_pool(name="rop", bufs=10))
    ms_pool = ctx.enter_context(tc.tile_pool(name="ms", bufs=2))
    pa_pool = ctx.enter_context(tc.tile_pool(name="pa", bufs=4, space="PSUM"))
    pss_pool = ctx.enter_context(tc.tile_pool(name="pss", bufs=2, space="PSUM"))
    pbc_pool = ctx.enter_context(tc.tile_pool(name="pbc", bufs=2, space="PSUM"))

    # x^T tiles [128 dim, T] from the gathered [2, T, 512] fp16 buffer via
    # DMA-transpose (XBAR path: 16-bit dtype, in rows %16, cols %128)
    xts = [xt_pool.tile([128, T], F16, tag="xt", name=f"xt{i}") for i in range(KD)]
    for kd in range(KD):
        half, dc = kd // 4, kd % 4
        src = xg[half * T:(half + 1) * T, dc * 128:(dc + 1) * 128]
        nc.sync.dma_start_transpose(xts[kd][:], src)

    w_qk = [wqk_pool.tile([128, FQK], F16, tag="wqk", name=f"wqk{i}") for i in range(KD)]
    w_v = [wv_pool.tile([128, FV], F16, tag="wv", name=f"wv{i}") for i in range(KD)]
    for kd in range(KD):
        nc.sync.dma_start(w_qk[kd][:], wqkg[kd * 128:(kd + 1) * 128, :])
        nc.sync.dma_start(w_v[kd][:], wvg[kd * 128:(kd + 1) * 128, :])

    for c in range(NCH):
        csl = slice(c * CS, (c + 1) * CS)

        # v for this chunk's 4 token sub-tiles: v = (x @ (lam0*wv)) + lam1*ve
        for sub in range(4):
            ti = c * 4 + sub
            tsl = slice(ti * 128, (ti + 1) * 128)
            pv = pa_pool.tile([128, FV], F32, tag="pa")
            for kd in range(KD):
                nc.tensor.matmul(
                    pv[:], xts[kd][:, tsl], w_v[kd][:],
                    start=(kd == 0), stop=(kd == KD - 1),
                )
            # ve tile straight out of the packed input (row-major view)
            ve_t = ve_pool.tile([128, FV], F16, tag="ve")
            nc.sync.dma_start(
                ve_t[:], dram["pk"].ap()[VE_OFF + ti * 64:VE_OFF + (ti + 1) * 64, :]
            )
            vtmp = ve_pool.tile([128, FV], BF16, tag="vt", bufs=2)
            nc.vector.tensor_scalar_mul(vtmp[:], ve_t[:], P["lam1c"][:])
            nc.vector.tensor_tensor(P["v_bf"][ti][:], vtmp[:], pv[:], ADD)

        # q/k per head: project, sumsq, rope
        rops = []
        rstds = []
        for h in range(HPG):
            for qi in range(2):
                f0 = h * 256 + qi * 128
                pqk = pa_pool.tile([128, CS], F32, tag="pa")
                for kd in range(KD):
                    nc.tensor.matmul(
                        pqk[:], w_qk[kd][:, f0:f0 + 128], xts[kd][:, csl],
                        start=(kd == 0), stop=(kd == KD - 1),
                    )
                raw = raw_pool.tile([128, CS], F16, tag="raw", bufs=3)
                nc.scalar.copy(raw[:], pqk[:])
                # mean of squares over the 128 head dims (partition axis):
                # Square(raw/sqrt(128)) summed by a ones matmul = mean
                sq = raw_pool.tile([128, CS], R32, tag="sq", bufs=3)
                nc.scalar.activation(
                    sq[:], raw[:], mybir.ActivationFunctionType.Square, scale=SCALE
                )
                ssps = pss_pool.tile([1, CS], F32, tag="ss")
                nc.tensor.matmul(ssps[:], P["ones_r"][:].bitcast(R32), sq[:],
                                 start=True, stop=True)
                ms_r = ms_pool.tile([1, CS], F32, tag="ms", bufs=4)
                nc.vector.tensor_scalar_add(ms_r[:], ssps[:], EPS)
                inv_r = ms_pool.tile([1, CS], F32, tag="inv", bufs=4)
                nc.vector.reciprocal(inv_r[:], ms_r[:])
                rstd = ms_pool.tile([1, CS], R32, tag="rstd", bufs=8,
                                    name=f"rstd{c}_{2 * h + qi}")
                nc.scalar.sqrt(rstd[:], inv_r[:])
                rstds.append(rstd)
                # rope: rows 0:64 = x1*c + x2*s ; rows 64:128 = x2*c - x1*s
                cos_t, sin_t = P["cos_t"], P["sin_t"]
                t_c1 = rt_pool.tile([64, CS], F16, tag="rt")
                t_s2 = rt_pool.tile([64, CS], F16, tag="rt")
                t_c2 = rt_pool.tile([64, CS], F16, tag="rt")
                t_s1 = rt_pool.tile([64, CS], F16, tag="rt")
                nc.vector.tensor_tensor(t_c1[:], raw[0:64, :], cos_t[0:64, csl], MUL)
                nc.vector.tensor_tensor(t_s2[:], raw[64:128, :], sin_t[64:128, csl], MUL)
                nc.vector.tensor_tensor(t_c2[:], raw[64:128, :], cos_t[64:128, csl], MUL)
                nc.vector.tensor_tensor(t_s1[:], raw[0:64, :], sin_t[0:64, csl], MUL)
                rop = rop_pool.tile([128, CS], F16, tag="rop")
                nc.vector.tensor_tensor(rop[0:64, :], t_c1[:], t_s2[:], ADD)
                nc.vector.tensor_tensor(rop[64:128, :], t_c2[:], t_s1[:], SUB)
                rops.append(rop)

        # normalize rope outputs into qkT
        for row in range(8):
            pbc = pbc_pool.tile([128, CS], F32, tag="bc")
            nc.tensor.matmul(
                pbc[:], P["ones1_r"][:].bitcast(R32), rstds[row][:],
                start=True, stop=True
            )
            nc.vector.tensor_tensor(P["qkT"][row][:, csl], rops[row][:], pbc[:], MUL)


def _phase_b(nc, tc, ctx, P):
    """Causal attention per head, scores^T layout."""
    ex_pool = ctx.enter_context(tc.tile_pool(name="exp", bufs=KT))
    sm_pool = ctx.enter_context(tc.tile_pool(name="sm", bufs=3))
    rb_pool = ctx.enter_context(tc.tile_pool(name="rb", bufs=2))
    pb_pool = ctx.enter_context(tc.tile_pool(name="pb", bufs=3, space="PSUM"))
    py_pool = ctx.enter_context(tc.tile_pool(name="py", bufs=2, space="PSUM"))
    pd_pool = ctx.enter_context(tc.tile_pool(name="pd", bufs=2, space="PSUM"))
    pn_pool = ctx.enter_context(tc.tile_pool(name="pn", bufs=1, space="PSUM"))

    for h in range(HPG):
        qh, kh = P["qkT"][2 * h], P["qkT"][2 * h + 1]
        for c in range(NCH):
            csl = slice(c * CS, (c + 1) * CS)
            nkt = 4 * (c + 1)
            exs = []
            for kt in range(nkt):
                ps = pb_pool.tile([128, CS], F32, tag="s")
                nc.tensor.matmul(
                    ps[:], kh[:, kt * 128:(kt + 1) * 128], qh[:, csl],
                    start=True, stop=True,
                )
                ex = ex_pool.tile([128, CS], BF16, tag="ex")
                nc.scalar.activation(
                    ex[:], ps[:], mybir.ActivationFunctionType.Exp, scale=SCALE
                )
                r = kt - 4 * c
                if r >= 0:
                    # keep where q-token j >= k-token (128*r + p): causal mask
                    nc.gpsimd.affine_select(
                        out=ex[:], in_=ex[:],
                        compare_op=mybir.AluOpType.is_ge, fill=0.0,
                        base=-128 * r, channel_multiplier=-1,
                        pattern=[[1, CS]],
                    )
                exs.append(ex)
            yac = py_pool.tile([128, CS], F32, tag="y")
            den = pd_pool.tile([1, CS], F32, tag="d")
            for kt in range(nkt):
                nc.tensor.matmul(
                    yac[:], P["v_bf"][kt][:, h * 128:(h + 1) * 128], exs[kt][:],
                    start=(kt == 0), stop=(kt == nkt - 1),
                )
            for kt in range(nkt):
                nc.tensor.matmul(
                    den[:], P["ones_b"][:], exs[kt][:],
                    start=(kt == 0), stop=(kt == nkt - 1),
                )
            rcp = sm_pool.tile([1, CS], R32, tag="rcp")
            nc.vector.reciprocal(rcp[:], den[:])
            pnb = pn_pool.tile([128, CS], F32, tag="nb")
            nc.tensor.matmul(pnb[:], P["ones1_r"][:].bitcast(R32), rcp[:],
                             start=True, stop=True)
            rbc = rb_pool.tile([128, CS], F32, tag="rb")
            nc.scalar.copy(rbc[:], pnb[:])
            nc.vector.tensor_tensor(P["yT"][h][:, csl], yac[:], rbc[:], MUL)


def _phase_c(nc, tc, ctx, P, cwg, opart):
    """c_proj partial straight into [token, dim] layout:
    o[t, m] = sum_j yT[j, t] * cwT[j, m]."""
    cw_pool = ctx.enter_context(tc.tile_pool(name="cwp", bufs=HPG))
    os_pool = ctx.enter_context(tc.tile_pool(name="os", bufs=4))
    pc_pool = ctx.enter_context(tc.tile_pool(name="pc", bufs=4, space="PSUM"))

    cwt = [cw_pool.tile([128, DIM], F16, tag="cw", name=f"cw{i}") for i in range(HPG)]
    for j in range(HPG):
        nc.sync.dma_start(cwt[j][:], cwg[j * 128:(j + 1) * 128, :])
    for ti in range(KT):
        tsl = slice(ti * 128, (ti + 1) * 128)
        for mo in range(2):
            msl = slice(mo * 512, (mo + 1) * 512)
            po = pc_pool.tile([128, 512], F32, tag="pc")
            for j in range(HPG):
                nc.tensor.matmul(
                    po[:], P["yT"][j][:, tsl], cwt[j][:, msl],
                    start=(j == 0), stop=(j == HPG - 1),
                )
            so = os_pool.tile([128, 512], F16, tag="os")
            nc.scalar.copy(so[:], po[:])
            nc.sync.dma_start(opart[tsl, msl], so[:])


def _build_program():
    nc = bacc.Bacc("TRN2", target_bir_lowering=False, debug=False, num_devices=B * HG)

    dram = {
        "pk": nc.dram_tensor("pk", [PK_ROWS, 1024], F16, kind="ExternalInput"),
        "oh": nc.dram_tensor("oh", [T // 2, DIM], F16, kind="ExternalOutput"),
    }

    with ExitStack() as top:
        top.enter_context(nc.allow_low_precision(reason="fp16/bf16 pipeline by design"))
        tc = top.enter_context(tile.TileContext(nc))
        dpool = top.enter_context(tc.tile_pool(name="dram", bufs=12, space="DRAM"))
        c_pool = top.enter_context(tc.tile_pool(name="const", bufs=1))
        qk_pool = top.enter_context(tc.tile_pool(name="qk", bufs=2 * HPG))
        v_pool = top.enter_context(tc.tile_pool(name="vbf", bufs=KT))
        y_pool = top.enter_context(tc.tile_pool(name="yt", bufs=HPG))

        # ---- on-device reconstruction of full tensors via collectives ----
        pk = dram["pk"].ap()
        xb = dpool.tile([T, XH], F16, name="xb")
        xg = dpool.tile([2 * T, XH], F16, name="xg")
        wqkb = dpool.tile([DIM // 4, FQK], F16, name="wqkb")
        wqkg = dpool.tile([DIM, FQK], F16, name="wqkg")
        wvb = dpool.tile([DIM // 4, FV], F16, name="wvb")
        wvg = dpool.tile([DIM, FV], F16, name="wvg")
        cwb = dpool.tile([FV // 4, DIM], F16, name="cwb")
        cwg = dpool.tile([FV, DIM], F16, name="cwg")
        csb = dpool.tile([16, 1024], F16, name="csb")
        csg = dpool.tile([128, 1024], F16, name="csg")
        opart = dpool.tile([T, DIM], F16, name="opart")
        oscat = dpool.tile([T // 2, DIM], F16, name="oscat")

        nc.gpsimd.dma_start(xb[:], pk[X_OFF:X_OFF + 1024, :])
        nc.gpsimd.dma_start(wqkb[:], pk[WQK_OFF:WQK_OFF + 256, :])
        nc.gpsimd.dma_start(wvb[:], pk[WV_OFF:WV_OFF + 128, :])
        nc.gpsimd.dma_start(cwb[:], pk[CW_OFF:CW_OFF + 128, :])
        nc.gpsimd.dma_start(csb[:], pk[CSN_OFF:CSN_OFF + 16, :])
        nc.gpsimd.collective_compute(
            "AllGather", mybir.AluOpType.bypass, replica_groups=PAIRS,
            ins=[xb.opt()], outs=[xg.opt()],
        )
        nc.gpsimd.collective_compute(
            "AllGather", mybir.AluOpType.bypass, replica_groups=QUADS,
            ins=[wqkb.opt()], outs=[wqkg.opt()],
        )
        nc.gpsimd.collective_compute(
            "AllGather", mybir.AluOpType.bypass, replica_groups=QUADS,
            ins=[wvb.opt()], outs=[wvg.opt()],
        )
        nc.gpsimd.collective_compute(
            "AllGather", mybir.AluOpType.bypass, replica_groups=QUADS,
            ins=[cwb.opt()], outs=[cwg.opt()],
        )
        nc.gpsimd.collective_compute(
            "AllGather", mybir.AluOpType.bypass, replica_groups=ALL8,
            ins=[csb.opt()], outs=[csg.opt()],
        )

        # ---- constants ----
        P = {
            "qkT": [qk_pool.tile([128, T], F16, tag="qk", name=f"qkT{i}")
                    for i in range(2 * HPG)],
            "v_bf": [v_pool.tile([128, FV], BF16, tag="v", name=f"vbf{i}")
                     for i in range(KT)],
            "yT": [y_pool.tile([128, T], F16, tag="y", name=f"yT{i}")
                   for i in range(HPG)],
            "ones_r": c_pool.tile([128, 1], F32, tag="ones", name="ones_r"),
            "ones1_r": c_pool.tile([1, 128], F32, tag="ones1", name="ones1_r"),
            "ones_b": c_pool.tile([128, 1], BF16, tag="onesbf", name="ones_b"),
            "cos_t": c_pool.tile([128, T], F16, tag="cos", name="cos_t"),
            "sin_t": c_pool.tile([128, T], F16, tag="sin", name="sin_t"),
            "lam1c": c_pool.tile([128, 1], F32, tag="lam1", name="lam1c"),
        }
        nc.vector.memset(P["ones_r"][:], 1.0)
        nc.vector.memset(P["ones1_r"][:], 1.0)
        nc.vector.memset(P["ones_b"][:], 1.0)
        # rope tables: rows 0:32 real freqs, 32:64 freq=0 (cos=1, sin=0),
        # duplicated at partition offset 64 for the two rope halves.
        # csg rows (width 1024): 0:64 = cos [32, 2048], 64:128 = sin [32, 2048]
        for off in (0, 64):
            nc.sync.dma_start(P["cos_t"][off:off + 32, :], csg[0:64, :])
            nc.sync.dma_start(P["sin_t"][off:off + 32, :], csg[64:128, :])
            nc.vector.memset(P["cos_t"][off + 32:off + 64, :], 1.0)
            nc.vector.memset(P["sin_t"][off + 32:off + 64, :], 0.0)
        # broadcast lambda_1 (an fp16 scalar in the packed input) to [128,1]
        with ExitStack() as lctx:
            lam_pool = lctx.enter_context(tc.tile_pool(name="lamp", bufs=1))
            pl_pool = lctx.enter_context(tc.tile_pool(name="pl", bufs=1, space="PSUM"))
            lam_t = lam_pool.tile([1, 2], F16, tag="lam", name="lam_t")
            lam_f = lam_pool.tile([1, 2], R32, tag="lamf", name="lam_f")
            nc.sync.dma_start(lam_t[:], pk[LAM_OFF:LAM_OFF + 1, 0:2])
            nc.vector.tensor_copy(lam_f[:], lam_t[:])
            plam = pl_pool.tile([128, 2], F32, tag="plam")
            nc.tensor.matmul(plam[:], P["ones1_r"][:].bitcast(R32),
                             lam_f[:], start=True, stop=True)
            nc.scalar.copy(P["lam1c"][:], plam[:, 0:1])

        with ExitStack() as ctx_a:
            _phase_a(nc, tc, ctx_a, dram, P, xg, wqkg, wvg)
        with ExitStack() as ctx_b:
            _phase_b(nc, tc, ctx_b, P)
        with ExitStack() as ctx_c:
            _phase_c(nc, tc, ctx_c, P, cwg, opart)

        nc.gpsimd.collective_compute(
            "ReduceScatter", mybir.AluOpType.add, replica_groups=PAIRS,
            ins=[opart.opt()], outs=[oscat.opt()],
        )
        nc.gpsimd.dma_start(dram["oh"].ap(), oscat[:])

    nc.compile()
    return nc


def _prep_inputs(x, ve, qkv_w, lambdas, c_proj_w):
    if "tables" not in _cache:
        _cache["tables"] = _rope_tables()
    cossin = _cache["tables"]                      # [64, T] fp16
    x16 = np.asarray(x, np.float32).astype(np.float16)
    ve16 = np.asarray(ve, np.float32).astype(np.float16)
    qw, kw, vw = qkv_w[0], qkv_w[1], qkv_w[2]

    wqkT, wvT, cwT = [], [], []
    for g in range(HG):
        heads = range(g * HPG, (g + 1) * HPG)
        rows = np.concatenate(
            [np.concatenate([qw[h * D:(h + 1) * D], kw[h * D:(h + 1) * D]])
             for h in heads]
        )                                    # [1024, DIM]
        wqkT.append(np.ascontiguousarray(rows.T).astype(np.float16))
        vcols = slice(g * FV, (g + 1) * FV)
        wvT.append(np.ascontiguousarray(
            (lambdas[0] * vw[vcols]).T).astype(np.float16))
        cwT.append(np.ascontiguousarray(c_proj_w[:, vcols].T).astype(np.float16))

    lamrow = np.zeros((1, 1024), np.float16)
    lamrow[0, 0] = np.float16(lambdas[1])

    in_maps = []
    for core in range(B * HG):
        b, g = core // HG, core % HG
        vcols = slice(g * FV, (g + 1) * FV)
        pk = np.concatenate([
            x16[b][:, g * XH:(g + 1) * XH].reshape(-1, 1024),
            wqkT[g][b * 256:(b + 1) * 256],
            wvT[g][b * 256:(b + 1) * 256].reshape(-1, 1024),
            cwT[g][b * 128:(b + 1) * 128],
            ve16[b][:, vcols].reshape(-1, 1024),
            cossin[core * 8:(core + 1) * 8].reshape(-1, 1024),
            lamrow,
        ], axis=0)
        in_maps.append({"pk": pk})
    return in_maps


def kernel(x, ve, qkv_w, lambdas, c_proj_w):
    x = np.asarray(x, np.float32)
    ve = np.asarray(ve, np.float32)
    qkv_w = np.asarray(qkv_w, np.float32).reshape(3, H * D, DIM)
    lambdas = np.asarray(lambdas, np.float32)
    c_proj_w = np.asarray(c_proj_w, np.float32)

    if "nc" not in _cache:
        _cache["nc"] = _build_program()
    nc = _cache["nc"]

    in_maps = _prep_inputs(x, ve, qkv_w, lambdas, c_proj_w)
    res = run_bass_kernel_spmd(nc, in_maps, list(range(B * HG))).results

    out = np.empty((B, T, DIM), np.float32)
    for b in range(B):
        out[b, :T // 2] = res[HG * b]["oh"]
        out[b, T // 2:] = res[HG * b + 1]["oh"]
    return out


# revision 11
# speedup vs baseline: 4.1399x; 1.1504x over previous
"""Causal self-attention (B=4, T=2048, DIM=1024, H=8, D=128) on 8 trn2 cores.

The axon tunnel moves ~45-50 MB/s with a per-buffer fixed cost, so wall-clock
is dominated by host<->device bytes and transfer count. This version:
  - ships ONE packed fp16 tensor per core (~5.3MB) instead of 8 tensors
    (inputs randn-scale, weights ~+-0.03: fp16-safe; fp8 was measured to
    breach the 2e-2 error budget);
  - x ships as per-core *halves* along DIM (no duplication across the two
    head-group cores of a batch); an on-device pair AllGather reconstructs it;
  - qkv/c_proj weights ship as per-core *quarters* along DIM; an on-device
    AllGather over the 4 batch-cores of each head-group reconstructs them;
  - rope tables (32 non-trivial freq rows) ship 1/8th per core, deduped by an
    8-core AllGather; causal mask and ones constants are generated on device;
  - the c_proj partial is pair-ReduceScattered on device, so each core ships
    back only its unique token-half of the final output in fp16.
Total ~42MB in + 16MB out vs 172MB + 64MB for the f32 no-collective version.

Core i handles batch b = i//2, head-group g = i%2 (4 heads each).
Compute per core: x^T via DMA-transpose, fused QKV (fp16 matmuls, fp32 PSUM),
per-head RMSnorm + RoPE, causal attention in scores^T layout (softmax
denominator via M=1 PE matmuls, mask via affine_select), lambda-mix of V with
ve (lambda0 folded into wv host-side, lambda1 applied on device), c_proj into
[token, dim] layout, pair ReduceScatter over token halves.
"""
import sys

sys.path.insert(0, "/opt/trn_rl_repo")

from contextlib import ExitStack

import numpy as np

import concourse.bass as bass  # noqa: F401
import concourse.mybir as mybir
import concourse.tile as tile
from concourse import bacc
from concourse.bass_utils import run_bass_kernel_spmd

B, T, DIM, H, D = 4, 2048, 1024, 8, 128
HG = 2              # head-groups (tensor-parallel factor per batch)
HPG = H // HG       # heads per core
CS = 512            # t-chunk size (PSUM fp32 bank = 512 cols)
NCH = T // CS       # 4 chunks
KT = T // 128       # 16 tk tiles
KD = DIM // 128     # 8 contraction tiles
FQK = HPG * 2 * 128  # 1024 qk feature cols per core
FV = HPG * 128       # 512 v cols per core
XH = DIM // 2        # per-core x half width
F32 = mybir.dt.float32
R32 = mybir.dt.float32r
F16 = mybir.dt.float16
BF16 = mybir.dt.bfloat16
U8 = mybir.dt.uint8
U16 = mybir.dt.uint16
SHR = mybir.AluOpType.logical_shift_right
SHL = mybir.AluOpType.logical_shift_left
BAND = mybir.AluOpType.bitwise_and
BOR = mybir.AluOpType.bitwise_or
EPS = float(np.finfo(np.float32).eps)
SCALE = float(D ** -0.5)
MUL = mybir.AluOpType.mult
ADD = mybir.AluOpType.add
SUB = mybir.AluOpType.subtract

PAIRS = [[2 * b, 2 * b + 1] for b in range(B)]          # x gather / out scatter
QUADS = [[g, 2 + g, 4 + g, 6 + g] for g in range(HG)]   # weight gather
ALL8 = [list(range(B * HG))]                             # rope-table gather

# fp16 packed input: row offsets in a [PK16_ROWS, 1024] fp16 tensor
WQK_OFF = 0                    # wqk quarter [256, 1024]
WV_OFF = 256                   # wv quarter  [256, 512] -> 128 rows
CW_OFF = WV_OFF + 128          # cw quarter  [128, 1024]
CSN_OFF = CW_OFF + 128         # cos|sin slice [8, 2048] -> 16 rows
LAM_OFF = CSN_OFF + 16         # lambda1 fp16 at [LAM_OFF, 0]
PK16_ROWS = LAM_OFF + 1
# 12-bit packed x/ve: row offsets in a [PK8_ROWS, 2048] uint8 tensor.
# Each fp16 value is rounded to 12 bits: hi byte (sign/exp/mant[9:8]) in the
# hi plane, mant[7:4] in a nibble plane pairing columns (j, j+256).
XHI_OFF = 0                    # x hi   [2048, 512] u8 -> 512 rows
XNIB_OFF = 512                 # x nib  [2048, 256] u8 -> 256 rows
VEHI_OFF = 768                 # ve hi  [2048, 512] u8 -> 512 rows
VENIB_OFF = 1280               # ve nib [2048, 256] u8 -> 256 rows
PK8_ROWS = 1536
# 12-bit packed output [768, 2048] u8: hi [1024,1024] -> 512 rows,
# nib [1024, 512] (cols paired (j, j+512)) -> 256 rows
OHI_OFF = 0
ONIB_OFF = 512
OH8_ROWS = 768

_cache = {}


def _rope_tables():
    freq = (1.0 / 1024.0) ** np.linspace(0.0, 1.0, D // 4, dtype=np.float64)
    theta = np.arange(T, dtype=np.float64)[:, None] * freq[None, :]  # [T, 32]
    cos = np.cos(theta).astype(np.float16).T.copy()  # [32, T]
    sin = np.sin(theta).astype(np.float16).T.copy()
    return np.concatenate([cos, sin])                # [64, T]


def _unpack12(nc, pool, hi8, nib8, w):
    """Reassemble fp16 bits from a [128,w] hi-byte tile and [128,w/2] nibble
    tile (cols paired (j, j+w/2)); returns a U16 tile (bitcast to F16 at use)."""
    a8 = pool.tile([128, w], U8, tag="a12")
    u = pool.tile([128, w], U16, tag="u12")
    h16 = pool.tile([128, w], U16, tag="h16")
    nc.vector.tensor_scalar(a8[:, 0:w // 2], nib8[:], 4, 4, SHR, SHL)
    nc.vector.tensor_scalar(a8[:, w // 2:w], nib8[:], 15, 4, BAND, SHL)
    nc.vector.tensor_copy(u[:], a8[:])
    nc.vector.tensor_copy(h16[:], hi8[:])
    nc.vector.tensor_scalar(h16[:], h16[:], 8, None, SHL)
    nc.vector.tensor_tensor(u[:], u[:], h16[:], BOR)
    return u


def _phase_a(nc, tc, ctx, dram, P, xg, wqkg, wvg):
    """x^T via DMA-transpose, QKV projection, v-mix, RMSnorm stats, RoPE."""
    xt_pool = ctx.enter_context(tc.tile_pool(name="xt", bufs=KD))
    wqk_pool = ctx.enter_context(tc.tile_pool(name="w", bufs=KD))
    wv_pool = ctx.enter_context(tc.tile_pool(name="wvp", bufs=KD))
    ve_pool = ctx.enter_context(tc.tile_pool(name="vep", bufs=2))
    raw_pool = ctx.enter_context(tc.tile_pool(name="raw", bufs=6))
    rt_pool = ctx.enter_context(tc.tile_pool(name="rtmp", bufs=4))
    rop_pool = ctx.enter_context(tc.tile_pool(name="rop", bufs=10))
    ms_pool = ctx.enter_context(tc.tile_pool(name="ms", bufs=2))
    pa_pool = ctx.enter_context(tc.tile_pool(name="pa", bufs=4, space="PSUM"))
    pss_pool = ctx.enter_context(tc.tile_pool(name="pss", bufs=2, space="PSUM"))
    pbc_pool = ctx.enter_context(tc.tile_pool(name="pbc", bufs=2, space="PSUM"))

    # x^T tiles [128 dim, T] from the gathered [2, T, 512] fp16 buffer via
    # DMA-transpose (XBAR path: 16-bit dtype, in rows %16, cols %128)
    xts = [xt_pool.tile([128, T], F16, tag="xt", name=f"xt{i}") for i in range(KD)]
    for kd in range(KD):
        half, dc = kd // 4, kd % 4
        src = xg[half * T:(half + 1) * T, dc * 128:(dc + 1) * 128]
        nc.sync.dma_start_transpose(xts[kd][:], src)

    w_qk = [wqk_pool.tile([128, FQK], F16, tag="wqk", name=f"wqk{i}") for i in range(KD)]
    w_v = [wv_pool.tile([128, FV], F16, tag="wv", name=f"wv{i}") for i in range(KD)]
    for kd in range(KD):
        nc.sync.dma_start(w_qk[kd][:], wqkg[kd * 128:(kd + 1) * 128, :])
        nc.sync.dma_start(w_v[kd][:], wvg[kd * 128:(kd + 1) * 128, :])

    for c in range(NCH):
        csl = slice(c * CS, (c + 1) * CS)

        # v for this chunk's 4 token sub-tiles: v = (x @ (lam0*wv)) + lam1*ve
        for sub in range(4):
            ti = c * 4 + sub
            tsl = slice(ti * 128, (ti + 1) * 128)
            pv = pa_pool.tile([128, FV], F32, tag="pa")
            for kd in range(KD):
                nc.tensor.matmul(
                    pv[:], xts[kd][:, tsl], w_v[kd][:],
                    start=(kd == 0), stop=(kd == KD - 1),
                )
            # ve tile: unpack 12-bit planes from the packed u8 input
            pk8 = dram["pk8"].ap()
            vh8 = ve_pool.tile([128, FV], U8, tag="vh8")
            vn8 = ve_pool.tile([128, FV // 2], U8, tag="vn8")
            nc.sync.dma_start(
                vh8[:], pk8[VEHI_OFF + ti * 32:VEHI_OFF + (ti + 1) * 32, :])
            nc.sync.dma_start(
                vn8[:], pk8[VENIB_OFF + ti * 16:VENIB_OFF + (ti + 1) * 16, :])
            ve_t = _unpack12(nc, ve_pool, vh8, vn8, FV)
            vtmp = ve_pool.tile([128, FV], BF16, tag="vt", bufs=2)
            nc.vector.tensor_scalar_mul(vtmp[:], ve_t[:].bitcast(F16), P["lam1c"][:])
            nc.vector.tensor_tensor(P["v_bf"][ti][:], vtmp[:], pv[:], ADD)

        # q/k per head: project, sumsq, rope
        rops = []
        rstds = []
        for h in range(HPG):
            for qi in range(2):
                f0 = h * 256 + qi * 128
                pqk = pa_pool.tile([128, CS], F32, tag="pa")
                for kd in range(KD):
                    nc.tensor.matmul(
                        pqk[:], w_qk[kd][:, f0:f0 + 128], xts[kd][:, csl],
                        start=(kd == 0), stop=(kd == KD - 1),
                    )
                raw = raw_pool.tile([128, CS], F16, tag="raw", bufs=3)
                nc.scalar.copy(raw[:], pqk[:])
                # mean of squares over the 128 head dims (partition axis):
                # Square(raw/sqrt(128)) summed by a ones matmul = mean
                sq = raw_pool.tile([128, CS], R32, tag="sq", bufs=3)
                nc.scalar.activation(
                    sq[:], raw[:], mybir.ActivationFunctionType.Square, scale=SCALE
                )
                ssps = pss_pool.tile([1, CS], F32, tag="ss")
                nc.tensor.matmul(ssps[:], P["ones_r"][:].bitcast(R32), sq[:],
                                 start=True, stop=True)
                ms_r = ms_pool.tile([1, CS], F32, tag="ms", bufs=4)
                nc.vector.tensor_scalar_add(ms_r[:], ssps[:], EPS)
                inv_r = ms_pool.tile([1, CS], F32, tag="inv", bufs=4)
                nc.vector.reciprocal(inv_r[:], ms_r[:])
                rstd = ms_pool.tile([1, CS], R32, tag="rstd", bufs=8,
                                    name=f"rstd{c}_{2 * h + qi}")
                nc.scalar.sqrt(rstd[:], inv_r[:])
                rstds.append(rstd)
                # rope: rows 0:64 = x1*c + x2*s ; rows 64:128 = x2*c - x1*s
                cos_t, sin_t = P["cos_t"], P["sin_t"]
                t_c1 = rt_pool.tile([64, CS], F16, tag="rt")
                t_s2 = rt_pool.tile([64, CS], F16, tag="rt")
                t_c2 = rt_pool.tile([64, CS], F16, tag="rt")
                t_s1 = rt_pool.tile([64, CS], F16, tag="rt")
                nc.vector.tensor_tensor(t_c1[:], raw[0:64, :], cos_t[0:64, csl], MUL)
                nc.vector.tensor_tensor(t_s2[:], raw[64:128, :], sin_t[64:128, csl], MUL)
                nc.vector.tensor_tensor(t_c2[:], raw[64:128, :], cos_t[64:128, csl], MUL)
                nc.vector.tensor_tensor(t_s1[:], raw[0:64, :], sin_t[0:64, csl], MUL)
                rop = rop_pool.tile([128, CS], F16, tag="rop")
                nc.vector.tensor_tensor(rop[0:64, :], t_c1[:], t_s2[:], ADD)
                nc.vector.tensor_tensor(rop[64:128, :], t_c2[:], t_s1[:], SUB)
                rops.append(rop)

        # normalize rope outputs into qkT
        for row in range(8):
            pbc = pbc_pool.tile([128, CS], F32, tag="bc")
            nc.tensor.matmul(
                pbc[:], P["ones1_r"][:].bitcast(R32), rstds[row][:],
                start=True, stop=True
            )
            nc.vector.tensor_tensor(P["qkT"][row][:, csl], rops[row][:], pbc[:], MUL)


def _phase_b(nc, tc, ctx, P):
    """Causal attention per head, scores^T layout."""
    ex_pool = ctx.enter_context(tc.tile_pool(name="exp", bufs=KT))
    sm_pool = ctx.enter_context(tc.tile_pool(name="sm", bufs=3))
    rb_pool = ctx.enter_context(tc.tile_pool(name="rb", bufs=2))
    pb_pool = ctx.enter_context(tc.tile_pool(name="pb", bufs=3, space="PSUM"))
    py_pool = ctx.enter_context(tc.tile_pool(name="py", bufs=2, space="PSUM"))
    pd_pool = ctx.enter_context(tc.tile_pool(name="pd", bufs=2, space="PSUM"))
    pn_pool = ctx.enter_context(tc.tile_pool(name="pn", bufs=1, space="PSUM"))

    for h in range(HPG):
        qh, kh = P["qkT"][2 * h], P["qkT"][2 * h + 1]
        for c in range(NCH):
            csl = slice(c * CS, (c + 1) * CS)
            nkt = 4 * (c + 1)
            exs = []
            for kt in range(nkt):
                ps = pb_pool.tile([128, CS], F32, tag="s")
                nc.tensor.matmul(
                    ps[:], kh[:, kt * 128:(kt + 1) * 128], qh[:, csl],
                    start=True, stop=True,
                )
                ex = ex_pool.tile([128, CS], BF16, tag="ex")
                nc.scalar.activation(
                    ex[:], ps[:], mybir.ActivationFunctionType.Exp, scale=SCALE
                )
                r = kt - 4 * c
                if r >= 0:
                    # keep where q-token j >= k-token (128*r + p): causal mask
                    nc.gpsimd.affine_select(
                        out=ex[:], in_=ex[:],
                        compare_op=mybir.AluOpType.is_ge, fill=0.0,
                        base=-128 * r, channel_multiplier=-1,
                        pattern=[[1, CS]],
                    )
                exs.append(ex)
            yac = py_pool.tile([128, CS], F32, tag="y")
            den = pd_pool.tile([1, CS], F32, tag="d")
            for kt in range(nkt):
                nc.tensor.matmul(
                    yac[:], P["v_bf"][kt][:, h * 128:(h + 1) * 128], exs[kt][:],
                    start=(kt == 0), stop=(kt == nkt - 1),
                )
            for kt in range(nkt):
                nc.tensor.matmul(
                    den[:], P["ones_b"][:], exs[kt][:],
                    start=(kt == 0), stop=(kt == nkt - 1),
                )
            rcp = sm_pool.tile([1, CS], R32, tag="rcp")
            nc.vector.reciprocal(rcp[:], den[:])
            pnb = pn_pool.tile([128, CS], F32, tag="nb")
            nc.tensor.matmul(pnb[:], P["ones1_r"][:].bitcast(R32), rcp[:],
                             start=True, stop=True)
            rbc = rb_pool.tile([128, CS], F32, tag="rb")
            nc.scalar.copy(rbc[:], pnb[:])
            nc.vector.tensor_tensor(P["yT"][h][:, csl], yac[:], rbc[:], MUL)


def _phase_c(nc, tc, ctx, P, cwg, opart):
    """c_proj partial straight into [token, dim] layout:
    o[t, m] = sum_j yT[j, t] * cwT[j, m]."""
    cw_pool = ctx.enter_context(tc.tile_pool(name="cwp", bufs=HPG))
    os_pool = ctx.enter_context(tc.tile_pool(name="os", bufs=4))
    pc_pool = ctx.enter_context(tc.tile_pool(name="pc", bufs=4, space="PSUM"))

    cwt = [cw_pool.tile([128, DIM], F16, tag="cw", name=f"cw{i}") for i in range(HPG)]
    for j in range(HPG):
        nc.sync.dma_start(cwt[j][:], cwg[j * 128:(j + 1) * 128, :])
    for ti in range(KT):
        tsl = slice(ti * 128, (ti + 1) * 128)
        for mo in range(2):
            msl = slice(mo * 512, (mo + 1) * 512)
            po = pc_pool.tile([128, 512], F32, tag="pc")
            for j in range(HPG):
                nc.tensor.matmul(
                    po[:], P["yT"][j][:, tsl], cwt[j][:, msl],
                    start=(j == 0), stop=(j == HPG - 1),
                )
            so = os_pool.tile([128, 512], F16, tag="os")
            nc.scalar.copy(so[:], po[:])
            nc.sync.dma_start(opart[tsl, msl], so[:])


def _build_program():
    nc = bacc.Bacc("TRN2", target_bir_lowering=False, debug=False, num_devices=B * HG)

    dram = {
        "pk16": nc.dram_tensor("pk16", [PK16_ROWS, 1024], F16, kind="ExternalInput"),
        "pk8": nc.dram_tensor("pk8", [PK8_ROWS, 2048], U8, kind="ExternalInput"),
        "oh8": nc.dram_tensor("oh8", [OH8_ROWS, 2048], U8, kind="ExternalOutput"),
    }

    with ExitStack() as top:
        top.enter_context(nc.allow_low_precision(reason="fp16/bf16 pipeline by design"))
        tc = top.enter_context(tile.TileContext(nc))
        dpool = top.enter_context(tc.tile_pool(name="dram", bufs=12, space="DRAM"))
        c_pool = top.enter_context(tc.tile_pool(name="const", bufs=1))
        qk_pool = top.enter_context(tc.tile_pool(name="qk", bufs=2 * HPG))
        v_pool = top.enter_context(tc.tile_pool(name="vbf", bufs=KT))
        y_pool = top.enter_context(tc.tile_pool(name="yt", bufs=HPG))

        # ---- on-device reconstruction of full tensors via collectives ----
        pk = dram["pk16"].ap()
        pk8 = dram["pk8"].ap()
        xb = dpool.tile([T, XH], F16, name="xb")
        xg = dpool.tile([2 * T, XH], F16, name="xg")
        wqkb = dpool.tile([DIM // 4, FQK], F16, name="wqkb")
        wqkg = dpool.tile([DIM, FQK], F16, name="wqkg")
        wvb = dpool.tile([DIM // 4, FV], F16, name="wvb")
        wvg = dpool.tile([DIM, FV], F16, name="wvg")
        cwb = dpool.tile([FV // 4, DIM], F16, name="cwb")
        cwg = dpool.tile([FV, DIM], F16, name="cwg")
        csb = dpool.tile([16, 1024], F16, name="csb")
        csg = dpool.tile([128, 1024], F16, name="csg")
        opart = dpool.tile([T, DIM], F16, name="opart")
        oscat = dpool.tile([T // 2, DIM], F16, name="oscat")

        nc.gpsimd.dma_start(wqkb[:], pk[WQK_OFF:WQK_OFF + 256, :])
        nc.gpsimd.dma_start(wvb[:], pk[WV_OFF:WV_OFF + 128, :])
        nc.gpsimd.dma_start(cwb[:], pk[CW_OFF:CW_OFF + 128, :])
        nc.gpsimd.dma_start(csb[:], pk[CSN_OFF:CSN_OFF + 16, :])
        # unpack this core's 12-bit x half into the fp16 bounce, then gather
        with ExitStack() as xctx:
            xup_pool = xctx.enter_context(tc.tile_pool(name="xup", bufs=4))
            for ti in range(KT):
                xh8 = xup_pool.tile([128, XH], U8, tag="xh8")
                xn8 = xup_pool.tile([128, XH // 2], U8, tag="xn8")
                nc.sync.dma_start(
                    xh8[:], pk8[XHI_OFF + ti * 32:XHI_OFF + (ti + 1) * 32, :])
                nc.sync.dma_start(
                    xn8[:], pk8[XNIB_OFF + ti * 16:XNIB_OFF + (ti + 1) * 16, :])
                xu = _unpack12(nc, xup_pool, xh8, xn8, XH)
                nc.sync.dma_start(
                    xb[ti * 128:(ti + 1) * 128, :], xu[:].bitcast(F16))
        nc.gpsimd.collective_compute(
            "AllGather", mybir.AluOpType.bypass, replica_groups=PAIRS,
            ins=[xb.opt()], outs=[xg.opt()],
        )
        nc.gpsimd.collective_compute(
            "AllGather", mybir.AluOpType.bypass, replica_groups=QUADS,
            ins=[wqkb.opt()], outs=[wqkg.opt()],
        )
        nc.gpsimd.collective_compute(
            "AllGather", mybir.AluOpType.bypass, replica_groups=QUADS,
            ins=[wvb.opt()], outs=[wvg.opt()],
        )
        nc.gpsimd.collective_compute(
            "AllGather", mybir.AluOpType.bypass, replica_groups=QUADS,
            ins=[cwb.opt()], outs=[cwg.opt()],
        )
        nc.gpsimd.collective_compute(
            "AllGather", mybir.AluOpType.bypass, replica_groups=ALL8,
            ins=[csb.opt()], outs=[csg.opt()],
        )

        # ---- constants ----
        P = {
            "qkT": [qk_pool.tile([128, T], F16, tag="qk", name=f"qkT{i}")
                    for i in range(2 * HPG)],
            "v_bf": [v_pool.tile([128, FV], BF16, tag="v", name=f"vbf{i}")
                     for i in range(KT)],
            "yT": [y_pool.tile([128, T], F16, tag="y", name=f"yT{i}")
                   for i in range(HPG)],
            "ones_r": c_pool.tile([128, 1], F32, tag="ones", name="ones_r"),
            "ones1_r": c_pool.tile([1, 128], F32, tag="ones1", name="ones1_r"),
            "ones_b": c_pool.tile([128, 1], BF16, tag="onesbf", name="ones_b"),
            "cos_t": c_pool.tile([128, T], F16, tag="cos", name="cos_t"),
            "sin_t": c_pool.tile([128, T], F16, tag="sin", name="sin_t"),
            "lam1c": c_pool.tile([128, 1], F32, tag="lam1", name="lam1c"),
        }
        nc.vector.memset(P["ones_r"][:], 1.0)
        nc.vector.memset(P["ones1_r"][:], 1.0)
        nc.vector.memset(P["ones_b"][:], 1.0)
        # rope tables: rows 0:32 real freqs, 32:64 freq=0 (cos=1, sin=0),
        # duplicated at partition offset 64 for the two rope halves.
        # csg rows (width 1024): 0:64 = cos [32, 2048], 64:128 = sin [32, 2048]
        for off in (0, 64):
            nc.sync.dma_start(P["cos_t"][off:off + 32, :], csg[0:64, :])
            nc.sync.dma_start(P["sin_t"][off:off + 32, :], csg[64:128, :])
            nc.vector.memset(P["cos_t"][off + 32:off + 64, :], 1.0)
            nc.vector.memset(P["sin_t"][off + 32:off + 64, :], 0.0)
        # broadcast lambda_1 (an fp16 scalar in the packed input) to [128,1]
        with ExitStack() as lctx:
            lam_pool = lctx.enter_context(tc.tile_pool(name="lamp", bufs=1))
            pl_pool = lctx.enter_context(tc.tile_pool(name="pl", bufs=1, space="PSUM"))
            lam_t = lam_pool.tile([1, 2], F16, tag="lam", name="lam_t")
            lam_f = lam_pool.tile([1, 2], R32, tag="lamf", name="lam_f")
            nc.sync.dma_start(lam_t[:], pk[LAM_OFF:LAM_OFF + 1, 0:2])
            nc.vector.tensor_copy(lam_f[:], lam_t[:])
            plam = pl_pool.tile([128, 2], F32, tag="plam")
            nc.tensor.matmul(plam[:], P["ones1_r"][:].bitcast(R32),
                             lam_f[:], start=True, stop=True)
            nc.scalar.copy(P["lam1c"][:], plam[:, 0:1])

        with ExitStack() as ctx_a:
            _phase_a(nc, tc, ctx_a, dram, P, xg, wqkg, wvg)
        with ExitStack() as ctx_b:
            _phase_b(nc, tc, ctx_b, P)
        with ExitStack() as ctx_c:
            _phase_c(nc, tc, ctx_c, P, cwg, opart)

        nc.gpsimd.collective_compute(
            "ReduceScatter", mybir.AluOpType.add, replica_groups=PAIRS,
            ins=[opart.opt()], outs=[oscat.opt()],
        )
        # pack the fp16 token-half output to 12 bits (round, split planes)
        with ExitStack() as octx:
            opk_pool = octx.enter_context(tc.tile_pool(name="opk", bufs=6))
            oh8 = dram["oh8"].ap()
            for t in range(8):
                ot16 = opk_pool.tile([128, DIM], F16, tag="ot")
                nc.sync.dma_start(ot16[:], oscat[t * 128:(t + 1) * 128, :])
                ur = opk_pool.tile([128, DIM], U16, tag="ur")
                nc.vector.tensor_scalar(ur[:], ot16[:].bitcast(U16), 8, None, ADD)
                hi16t = opk_pool.tile([128, DIM], U16, tag="ohi16")
                nc.vector.tensor_scalar(hi16t[:], ur[:], 8, None, SHR)
                hi8t = opk_pool.tile([128, DIM], U8, tag="ohi")
                nc.vector.tensor_copy(hi8t[:], hi16t[:])
                lo16t = opk_pool.tile([128, DIM], U16, tag="olo16")
                nc.vector.tensor_scalar(lo16t[:], ur[:], 4, 15, SHR, BAND)
                lo4t = opk_pool.tile([128, DIM], U8, tag="olo")
                nc.vector.tensor_copy(lo4t[:], lo16t[:])
                nibt = opk_pool.tile([128, DIM // 2], U8, tag="onib")
                nc.vector.tensor_scalar(nibt[:], lo4t[:, 0:DIM // 2], 4, None, SHL)
                nc.vector.tensor_tensor(nibt[:], nibt[:], lo4t[:, DIM // 2:DIM], BOR)
                nc.sync.dma_start(oh8[OHI_OFF + t * 64:OHI_OFF + (t + 1) * 64, :],
                                  hi8t[:])
                nc.sync.dma_start(oh8[ONIB_OFF + t * 32:ONIB_OFF + (t + 1) * 32, :],
                                  nibt[:])

    nc.compile()
    return nc


def _planes12(a32):
    """[B, T, DIM] f32 -> (hi [B,T,DIM] u8, lo4 [B,T,DIM] u8) of the fp16
    values rounded to 12 bits (round-half-up on the dropped 4 mantissa bits)."""
    u = a32.astype(np.float16).view(np.uint16) + np.uint16(8)
    hi = (u >> 8).astype(np.uint8)
    lo4 = ((u >> 4) & np.uint16(15)).astype(np.uint8)
    return hi, lo4


def _prep_inputs(x, ve, qkv_w, lambdas, c_proj_w):
    if "tables" not in _cache:
        _cache["tables"] = _rope_tables()
    cossin = _cache["tables"]                      # [64, T] fp16
    xhi, xlo = _planes12(np.asarray(x, np.float32))
    vhi, vlo = _planes12(np.asarray(ve, np.float32))
    qw, kw, vw = qkv_w[0], qkv_w[1], qkv_w[2]

    wqkT, wvT, cwT = [], [], []
    for g in range(HG):
        heads = range(g * HPG, (g + 1) * HPG)
        rows = np.concatenate(
            [np.concatenate([qw[h * D:(h + 1) * D], kw[h * D:(h + 1) * D]])
             for h in heads]
        )                                    # [1024, DIM]
        wqkT.append(np.ascontiguousarray(rows.T).astype(np.float16))
        vcols = slice(g * FV, (g + 1) * FV)
        wvT.append(np.ascontiguousarray(
            (lambdas[0] * vw[vcols]).T).astype(np.float16))
        cwT.append(np.ascontiguousarray(c_proj_w[:, vcols].T).astype(np.float16))

    lamrow = np.zeros((1, 1024), np.float16)
    lamrow[0, 0] = np.float16(lambdas[1])

    in_maps = []
    for core in range(B * HG):
        b, g = core // HG, core % HG
        cols = slice(g * XH, (g + 1) * XH)
        pk16 = np.concatenate([
            wqkT[g][b * 256:(b + 1) * 256],
            wvT[g][b * 256:(b + 1) * 256].reshape(-1, 1024),
            cwT[g][b * 128:(b + 1) * 128],
            cossin[core * 8:(core + 1) * 8].reshape(-1, 1024),
            lamrow,
        ], axis=0)
        xl = xlo[b][:, cols]
        vl = vlo[b][:, cols]
        pk8 = np.concatenate([
            xhi[b][:, cols].reshape(-1, 2048),
            ((xl[:, 0:256] << 4) | xl[:, 256:512]).reshape(-1, 2048),
            vhi[b][:, cols].reshape(-1, 2048),
            ((vl[:, 0:256] << 4) | vl[:, 256:512]).reshape(-1, 2048),
        ], axis=0)
        in_maps.append({"pk16": pk16, "pk8": pk8})
    return in_maps


def kernel(x, ve, qkv_w, lambdas, c_proj_w):
    x = np.asarray(x, np.float32)
    ve = np.asarray(ve, np.float32)
    qkv_w = np.asarray(qkv_w, np.float32).reshape(3, H * D, DIM)
    lambdas = np.asarray(lambdas, np.float32)
    c_proj_w = np.asarray(c_proj_w, np.float32)

    if "nc" not in _cache:
        _cache["nc"] = _build_program()
    nc = _cache["nc"]

    in_maps = _prep_inputs(x, ve, qkv_w, lambdas, c_proj_w)
    res = run_bass_kernel_spmd(nc, in_maps, list(range(B * HG))).results

    out = np.empty((B, T, DIM), np.float32)
    for b in range(B):
        for g in range(HG):
            o8 = res[HG * b + g]["oh8"]
            hi = o8[OHI_OFF:OHI_OFF + 512].reshape(T // 2, DIM)
            nib = o8[ONIB_OFF:ONIB_OFF + 256].reshape(T // 2, DIM // 2)
            u = hi.astype(np.uint16) << 8
            u[:, 0:DIM // 2] |= (nib >> 4).astype(np.uint16) << 4
            u[:, DIM // 2:] |= (nib & np.uint8(15)).astype(np.uint16) << 4
            out[b, g * (T // 2):(g + 1) * (T // 2)] = u.view(np.float16)
    return out


# revision 14
# speedup vs baseline: 4.4035x; 1.0637x over previous
"""Causal self-attention (B=4, T=2048, DIM=1024, H=8, D=128) on 8 trn2 cores.

The axon tunnel moves ~45-50 MB/s with a per-buffer fixed cost, so wall-clock
is dominated by host<->device bytes and transfer count. This version:
  - ships ONE packed fp16 tensor per core (~5.3MB) instead of 8 tensors
    (inputs randn-scale, weights ~+-0.03: fp16-safe; fp8 was measured to
    breach the 2e-2 error budget);
  - x ships as per-core *halves* along DIM (no duplication across the two
    head-group cores of a batch); an on-device pair AllGather reconstructs it;
  - qkv/c_proj weights ship as per-core *quarters* along DIM; an on-device
    AllGather over the 4 batch-cores of each head-group reconstructs them;
  - rope tables (32 non-trivial freq rows) ship 1/8th per core, deduped by an
    8-core AllGather; causal mask and ones constants are generated on device;
  - the c_proj partial is pair-ReduceScattered on device, so each core ships
    back only its unique token-half of the final output in fp16.
Total ~42MB in + 16MB out vs 172MB + 64MB for the f32 no-collective version.

Core i handles batch b = i//2, head-group g = i%2 (4 heads each).
Compute per core: x^T via DMA-transpose, fused QKV (fp16 matmuls, fp32 PSUM),
per-head RMSnorm + RoPE, causal attention in scores^T layout (softmax
denominator via M=1 PE matmuls, mask via affine_select), lambda-mix of V with
ve (lambda0 folded into wv host-side, lambda1 applied on device), c_proj into
[token, dim] layout, pair ReduceScatter over token halves.
"""
import sys

sys.path.insert(0, "/opt/trn_rl_repo")

from contextlib import ExitStack

import numpy as np

import concourse.bass as bass  # noqa: F401
import concourse.mybir as mybir
import concourse.tile as tile
from concourse import bacc
from concourse.bass_utils import run_bass_kernel_spmd

B, T, DIM, H, D = 4, 2048, 1024, 8, 128
HG = 2              # head-groups (tensor-parallel factor per batch)
HPG = H // HG       # heads per core
CS = 512            # t-chunk size (PSUM fp32 bank = 512 cols)
NCH = T // CS       # 4 chunks
KT = T // 128       # 16 tk tiles
KD = DIM // 128     # 8 contraction tiles
FQK = HPG * 2 * 128  # 1024 qk feature cols per core
FV = HPG * 128       # 512 v cols per core
XH = DIM // 2        # per-core x half width
F32 = mybir.dt.float32
R32 = mybir.dt.float32r
F16 = mybir.dt.float16
BF16 = mybir.dt.bfloat16
U8 = mybir.dt.uint8
U16 = mybir.dt.uint16
SHR = mybir.AluOpType.logical_shift_right
SHL = mybir.AluOpType.logical_shift_left
BAND = mybir.AluOpType.bitwise_and
BOR = mybir.AluOpType.bitwise_or
EPS = float(np.finfo(np.float32).eps)
SCALE = float(D ** -0.5)
MUL = mybir.AluOpType.mult
ADD = mybir.AluOpType.add
SUB = mybir.AluOpType.subtract

PAIRS = [[2 * b, 2 * b + 1] for b in range(B)]          # x gather / out scatter
QUADS = [[g, 2 + g, 4 + g, 6 + g] for g in range(HG)]   # weight gather
ALL8 = [list(range(B * HG))]                             # rope-table gather

# fp16 packed input: row offsets in a [PK16_ROWS, 1024] fp16 tensor
WQK_OFF = 0                    # wqk quarter [256, 1024]
WV_OFF = 256                   # wv quarter  [256, 512] -> 128 rows
CW_OFF = WV_OFF + 128          # cw quarter  [128, 1024]
CSN_OFF = CW_OFF + 128         # cos|sin slice [8, 2048] -> 16 rows
LAM_OFF = CSN_OFF + 16         # lambda1 fp16 at [LAM_OFF, 0]
PK16_ROWS = LAM_OFF + 1
# 12-bit packed x/ve: row offsets in a [PK8_ROWS, 2048] uint8 tensor.
# Each fp16 value is rounded to 12 bits: hi byte (sign/exp/mant[9:8]) in the
# hi plane, mant[7:4] in a nibble plane pairing columns (j, j+256).
XHI_OFF = 0                    # x hi   [2048, 512] u8 -> 512 rows
XNIB_OFF = 512                 # x nib  [2048, 256] u8 -> 256 rows
VEHI_OFF = 768                 # ve hi  [2048, 512] u8 -> 512 rows
VENIB_OFF = 1280               # ve nib [2048, 256] u8 -> 256 rows
PK16_BASE = 1536               # fp16 region viewed as u8 rows (1 row = 1024 f16)
PK8_ROWS = PK16_BASE + PK16_ROWS
# 12-bit packed output [768, 2048] u8: hi [1024,1024] -> 512 rows,
# nib [1024, 512] (cols paired (j, j+512)) -> 256 rows
OHI_OFF = 0
ONIB_OFF = 512
OH8_ROWS = 768

_cache = {}


def _rope_tables():
    freq = (1.0 / 1024.0) ** np.linspace(0.0, 1.0, D // 4, dtype=np.float64)
    theta = np.arange(T, dtype=np.float64)[:, None] * freq[None, :]  # [T, 32]
    cos = np.cos(theta).astype(np.float16).T.copy()  # [32, T]
    sin = np.sin(theta).astype(np.float16).T.copy()
    return np.concatenate([cos, sin])                # [64, T]


def _unpack12(nc, pool, hi8, nib8, w):
    """Reassemble fp16 bits from a [128,w] hi-byte tile and [128,w/2] nibble
    tile (cols paired (j, j+w/2)); returns a U16 tile (bitcast to F16 at use)."""
    a8 = pool.tile([128, w], U8, tag="a12")
    u = pool.tile([128, w], U16, tag="u12")
    h16 = pool.tile([128, w], U16, tag="h16")
    nc.vector.tensor_scalar(a8[:, 0:w // 2], nib8[:], 4, 4, SHR, SHL)
    nc.vector.tensor_scalar(a8[:, w // 2:w], nib8[:], 15, 4, BAND, SHL)
    nc.vector.tensor_copy(u[:], a8[:])
    nc.vector.tensor_copy(h16[:], hi8[:])
    nc.vector.tensor_scalar(h16[:], h16[:], 8, None, SHL)
    nc.vector.tensor_tensor(u[:], u[:], h16[:], BOR)
    return u


def _phase_a(nc, tc, ctx, dram, P, xg, wqkg, wvg):
    """x^T via DMA-transpose, QKV projection, v-mix, RMSnorm stats, RoPE."""
    xt_pool = ctx.enter_context(tc.tile_pool(name="xt", bufs=KD))
    wqk_pool = ctx.enter_context(tc.tile_pool(name="w", bufs=KD))
    wv_pool = ctx.enter_context(tc.tile_pool(name="wvp", bufs=KD))
    ve_pool = ctx.enter_context(tc.tile_pool(name="vep", bufs=2))
    raw_pool = ctx.enter_context(tc.tile_pool(name="raw", bufs=6))
    rt_pool = ctx.enter_context(tc.tile_pool(name="rtmp", bufs=4))
    rop_pool = ctx.enter_context(tc.tile_pool(name="rop", bufs=10))
    ms_pool = ctx.enter_context(tc.tile_pool(name="ms", bufs=2))
    pa_pool = ctx.enter_context(tc.tile_pool(name="pa", bufs=4, space="PSUM"))
    pss_pool = ctx.enter_context(tc.tile_pool(name="pss", bufs=2, space="PSUM"))
    pbc_pool = ctx.enter_context(tc.tile_pool(name="pbc", bufs=2, space="PSUM"))

    # x^T tiles [128 dim, T] from the gathered [2, T, 512] fp16 buffer via
    # DMA-transpose (XBAR path: 16-bit dtype, in rows %16, cols %128)
    xts = [xt_pool.tile([128, T], F16, tag="xt", name=f"xt{i}") for i in range(KD)]
    for kd in range(KD):
        half, dc = kd // 4, kd % 4
        src = xg[half * T:(half + 1) * T, dc * 128:(dc + 1) * 128]
        nc.sync.dma_start_transpose(xts[kd][:], src)

    w_qk = [wqk_pool.tile([128, FQK], F16, tag="wqk", name=f"wqk{i}") for i in range(KD)]
    w_v = [wv_pool.tile([128, FV], F16, tag="wv", name=f"wv{i}") for i in range(KD)]
    for kd in range(KD):
        nc.sync.dma_start(w_qk[kd][:], wqkg[kd * 128:(kd + 1) * 128, :])
        nc.sync.dma_start(w_v[kd][:], wvg[kd * 128:(kd + 1) * 128, :])

    for c in range(NCH):
        csl = slice(c * CS, (c + 1) * CS)

        # v for this chunk's 4 token sub-tiles: v = (x @ (lam0*wv)) + lam1*ve
        for sub in range(4):
            ti = c * 4 + sub
            tsl = slice(ti * 128, (ti + 1) * 128)
            pv = pa_pool.tile([128, FV], F32, tag="pa")
            for kd in range(KD):
                nc.tensor.matmul(
                    pv[:], xts[kd][:, tsl], w_v[kd][:],
                    start=(kd == 0), stop=(kd == KD - 1),
                )
            # ve tile: unpack 12-bit planes from the packed u8 input
            pk8 = dram["pk8"].ap()
            vh8 = ve_pool.tile([128, FV], U8, tag="vh8")
            vn8 = ve_pool.tile([128, FV // 2], U8, tag="vn8")
            nc.sync.dma_start(
                vh8[:], pk8[VEHI_OFF + ti * 32:VEHI_OFF + (ti + 1) * 32, :])
            nc.sync.dma_start(
                vn8[:], pk8[VENIB_OFF + ti * 16:VENIB_OFF + (ti + 1) * 16, :])
            ve_t = _unpack12(nc, ve_pool, vh8, vn8, FV)
            vtmp = ve_pool.tile([128, FV], BF16, tag="vt", bufs=2)
            nc.vector.tensor_scalar_mul(vtmp[:], ve_t[:].bitcast(F16), P["lam1c"][:])
            nc.vector.tensor_tensor(P["v_bf"][ti][:], vtmp[:], pv[:], ADD)

        # q/k per head: project, sumsq, rope
        rops = []
        rstds = []
        for h in range(HPG):
            for qi in range(2):
                f0 = h * 256 + qi * 128
                pqk = pa_pool.tile([128, CS], F32, tag="pa")
                for kd in range(KD):
                    nc.tensor.matmul(
                        pqk[:], w_qk[kd][:, f0:f0 + 128], xts[kd][:, csl],
                        start=(kd == 0), stop=(kd == KD - 1),
                    )
                raw = raw_pool.tile([128, CS], F16, tag="raw", bufs=3)
                nc.scalar.copy(raw[:], pqk[:])
                # mean of squares over the 128 head dims (partition axis):
                # Square(raw/sqrt(128)) summed by a ones matmul = mean
                sq = raw_pool.tile([128, CS], R32, tag="sq", bufs=3)
                nc.scalar.activation(
                    sq[:], raw[:], mybir.ActivationFunctionType.Square, scale=SCALE
                )
                ssps = pss_pool.tile([1, CS], F32, tag="ss")
                nc.tensor.matmul(ssps[:], P["ones_r"][:].bitcast(R32), sq[:],
                                 start=True, stop=True)
                ms_r = ms_pool.tile([1, CS], F32, tag="ms", bufs=4)
                nc.vector.tensor_scalar_add(ms_r[:], ssps[:], EPS)
                inv_r = ms_pool.tile([1, CS], F32, tag="inv", bufs=4)
                nc.vector.reciprocal(inv_r[:], ms_r[:])
                rstd = ms_pool.tile([1, CS], R32, tag="rstd", bufs=8,
                                    name=f"rstd{c}_{2 * h + qi}")
                nc.scalar.sqrt(rstd[:], inv_r[:])
                rstds.append(rstd)
                # rope: rows 0:64 = x1*c + x2*s ; rows 64:128 = x2*c - x1*s
                cos_t, sin_t = P["cos_t"], P["sin_t"]
                t_c1 = rt_pool.tile([64, CS], F16, tag="rt")
                t_s2 = rt_pool.tile([64, CS], F16, tag="rt")
                t_c2 = rt_pool.tile([64, CS], F16, tag="rt")
                t_s1 = rt_pool.tile([64, CS], F16, tag="rt")
                nc.vector.tensor_tensor(t_c1[:], raw[0:64, :], cos_t[0:64, csl], MUL)
                nc.vector.tensor_tensor(t_s2[:], raw[64:128, :], sin_t[64:128, csl], MUL)
                nc.vector.tensor_tensor(t_c2[:], raw[64:128, :], cos_t[64:128, csl], MUL)
                nc.vector.tensor_tensor(t_s1[:], raw[0:64, :], sin_t[0:64, csl], MUL)
                rop = rop_pool.tile([128, CS], F16, tag="rop")
                nc.vector.tensor_tensor(rop[0:64, :], t_c1[:], t_s2[:], ADD)
                nc.vector.tensor_tensor(rop[64:128, :], t_c2[:], t_s1[:], SUB)
                rops.append(rop)

        # normalize rope outputs into qkT
        for row in range(8):
            pbc = pbc_pool.tile([128, CS], F32, tag="bc")
            nc.tensor.matmul(
                pbc[:], P["ones1_r"][:].bitcast(R32), rstds[row][:],
                start=True, stop=True
            )
            nc.vector.tensor_tensor(P["qkT"][row][:, csl], rops[row][:], pbc[:], MUL)


def _phase_b(nc, tc, ctx, P):
    """Causal attention per head, scores^T layout."""
    ex_pool = ctx.enter_context(tc.tile_pool(name="exp", bufs=KT))
    sm_pool = ctx.enter_context(tc.tile_pool(name="sm", bufs=3))
    rb_pool = ctx.enter_context(tc.tile_pool(name="rb", bufs=2))
    pb_pool = ctx.enter_context(tc.tile_pool(name="pb", bufs=3, space="PSUM"))
    py_pool = ctx.enter_context(tc.tile_pool(name="py", bufs=2, space="PSUM"))
    pd_pool = ctx.enter_context(tc.tile_pool(name="pd", bufs=2, space="PSUM"))
    pn_pool = ctx.enter_context(tc.tile_pool(name="pn", bufs=1, space="PSUM"))

    for h in range(HPG):
        qh, kh = P["qkT"][2 * h], P["qkT"][2 * h + 1]
        for c in range(NCH):
            csl = slice(c * CS, (c + 1) * CS)
            nkt = 4 * (c + 1)
            exs = []
            for kt in range(nkt):
                ps = pb_pool.tile([128, CS], F32, tag="s")
                nc.tensor.matmul(
                    ps[:], kh[:, kt * 128:(kt + 1) * 128], qh[:, csl],
                    start=True, stop=True,
                )
                ex = ex_pool.tile([128, CS], BF16, tag="ex")
                nc.scalar.activation(
                    ex[:], ps[:], mybir.ActivationFunctionType.Exp, scale=SCALE
                )
                r = kt - 4 * c
                if r >= 0:
                    # keep where q-token j >= k-token (128*r + p): causal mask
                    nc.gpsimd.affine_select(
                        out=ex[:], in_=ex[:],
                        compare_op=mybir.AluOpType.is_ge, fill=0.0,
                        base=-128 * r, channel_multiplier=-1,
                        pattern=[[1, CS]],
                    )
                exs.append(ex)
            yac = py_pool.tile([128, CS], F32, tag="y")
            den = pd_pool.tile([1, CS], F32, tag="d")
            for kt in range(nkt):
                nc.tensor.matmul(
                    yac[:], P["v_bf"][kt][:, h * 128:(h + 1) * 128], exs[kt][:],
                    start=(kt == 0), stop=(kt == nkt - 1),
                )
            for kt in range(nkt):
                nc.tensor.matmul(
                    den[:], P["ones_b"][:], exs[kt][:],
                    start=(kt == 0), stop=(kt == nkt - 1),
                )
            rcp = sm_pool.tile([1, CS], R32, tag="rcp")
            nc.vector.reciprocal(rcp[:], den[:])
            pnb = pn_pool.tile([128, CS], F32, tag="nb")
            nc.tensor.matmul(pnb[:], P["ones1_r"][:].bitcast(R32), rcp[:],
                             start=True, stop=True)
            rbc = rb_pool.tile([128, CS], F32, tag="rb")
            nc.scalar.copy(rbc[:], pnb[:])
            nc.vector.tensor_tensor(P["yT"][h][:, csl], yac[:], rbc[:], MUL)


def _phase_c(nc, tc, ctx, P, cwg, opart):
    """c_proj partial straight into [token, dim] layout:
    o[t, m] = sum_j yT[j, t] * cwT[j, m]."""
    cw_pool = ctx.enter_context(tc.tile_pool(name="cwp", bufs=HPG))
    os_pool = ctx.enter_context(tc.tile_pool(name="os", bufs=4))
    pc_pool = ctx.enter_context(tc.tile_pool(name="pc", bufs=4, space="PSUM"))

    cwt = [cw_pool.tile([128, DIM], F16, tag="cw", name=f"cw{i}") for i in range(HPG)]
    for j in range(HPG):
        nc.sync.dma_start(cwt[j][:], cwg[j * 128:(j + 1) * 128, :])
    for ti in range(KT):
        tsl = slice(ti * 128, (ti + 1) * 128)
        for mo in range(2):
            msl = slice(mo * 512, (mo + 1) * 512)
            po = pc_pool.tile([128, 512], F32, tag="pc")
            for j in range(HPG):
                nc.tensor.matmul(
                    po[:], P["yT"][j][:, tsl], cwt[j][:, msl],
                    start=(j == 0), stop=(j == HPG - 1),
                )
            so = os_pool.tile([128, 512], F16, tag="os")
            nc.scalar.copy(so[:], po[:])
            nc.sync.dma_start(opart[tsl, msl], so[:])


def _build_program():
    nc = bacc.Bacc("TRN2", target_bir_lowering=False, debug=False, num_devices=B * HG)

    dram = {
        "pk8": nc.dram_tensor("pk8", [PK8_ROWS, 2048], U8, kind="ExternalInput"),
        "oh8": nc.dram_tensor("oh8", [OH8_ROWS, 2048], U8, kind="ExternalOutput"),
    }

    with ExitStack() as top:
        top.enter_context(nc.allow_low_precision(reason="fp16/bf16 pipeline by design"))
        tc = top.enter_context(tile.TileContext(nc))
        dpool = top.enter_context(tc.tile_pool(name="dram", bufs=12, space="DRAM"))
        c_pool = top.enter_context(tc.tile_pool(name="const", bufs=1))
        qk_pool = top.enter_context(tc.tile_pool(name="qk", bufs=2 * HPG))
        v_pool = top.enter_context(tc.tile_pool(name="vbf", bufs=KT))
        y_pool = top.enter_context(tc.tile_pool(name="yt", bufs=HPG))

        # ---- on-device reconstruction of full tensors via collectives ----
        pk8 = dram["pk8"].ap()
        pk = pk8[PK16_BASE:PK16_BASE + PK16_ROWS, :].bitcast(F16)
        xb = dpool.tile([T, XH], F16, name="xb")
        xg = dpool.tile([2 * T, XH], F16, name="xg")
        wqkb = dpool.tile([DIM // 4, FQK], F16, name="wqkb")
        wqkg = dpool.tile([DIM, FQK], F16, name="wqkg")
        wvb = dpool.tile([DIM // 4, FV], F16, name="wvb")
        wvg = dpool.tile([DIM, FV], F16, name="wvg")
        cwb = dpool.tile([FV // 4, DIM], F16, name="cwb")
        cwg = dpool.tile([FV, DIM], F16, name="cwg")
        csb = dpool.tile([16, 1024], F16, name="csb")
        csg = dpool.tile([128, 1024], F16, name="csg")
        opart = dpool.tile([T, DIM], F16, name="opart")
        oscat = dpool.tile([T // 2, DIM], F16, name="oscat")

        nc.gpsimd.dma_start(wqkb[:], pk[WQK_OFF:WQK_OFF + 256, :])
        nc.gpsimd.dma_start(wvb[:], pk[WV_OFF:WV_OFF + 128, :])
        nc.gpsimd.dma_start(cwb[:], pk[CW_OFF:CW_OFF + 128, :])
        nc.gpsimd.dma_start(csb[:], pk[CSN_OFF:CSN_OFF + 16, :])
        # unpack this core's 12-bit x half into the fp16 bounce, then gather
        with ExitStack() as xctx:
            xup_pool = xctx.enter_context(tc.tile_pool(name="xup", bufs=4))
            for ti in range(KT):
                xh8 = xup_pool.tile([128, XH], U8, tag="xh8")
                xn8 = xup_pool.tile([128, XH // 2], U8, tag="xn8")
                nc.sync.dma_start(
                    xh8[:], pk8[XHI_OFF + ti * 32:XHI_OFF + (ti + 1) * 32, :])
                nc.sync.dma_start(
                    xn8[:], pk8[XNIB_OFF + ti * 16:XNIB_OFF + (ti + 1) * 16, :])
                xu = _unpack12(nc, xup_pool, xh8, xn8, XH)
                nc.sync.dma_start(
                    xb[ti * 128:(ti + 1) * 128, :], xu[:].bitcast(F16))
        nc.gpsimd.collective_compute(
            "AllGather", mybir.AluOpType.bypass, replica_groups=PAIRS,
            ins=[xb.opt()], outs=[xg.opt()],
        )
        nc.gpsimd.collective_compute(
            "AllGather", mybir.AluOpType.bypass, replica_groups=QUADS,
            ins=[wqkb.opt()], outs=[wqkg.opt()],
        )
        nc.gpsimd.collective_compute(
            "AllGather", mybir.AluOpType.bypass, replica_groups=QUADS,
            ins=[wvb.opt()], outs=[wvg.opt()],
        )
        nc.gpsimd.collective_compute(
            "AllGather", mybir.AluOpType.bypass, replica_groups=QUADS,
            ins=[cwb.opt()], outs=[cwg.opt()],
        )
        nc.gpsimd.collective_compute(
            "AllGather", mybir.AluOpType.bypass, replica_groups=ALL8,
            ins=[csb.opt()], outs=[csg.opt()],
        )

        # ---- constants ----
        P = {
            "qkT": [qk_pool.tile([128, T], F16, tag="qk", name=f"qkT{i}")
                    for i in range(2 * HPG)],
            "v_bf": [v_pool.tile([128, FV], BF16, tag="v", name=f"vbf{i}")
                     for i in range(KT)],
            "yT": [y_pool.tile([128, T], F16, tag="y", name=f"yT{i}")
                   for i in range(HPG)],
            "ones_r": c_pool.tile([128, 1], F32, tag="ones", name="ones_r"),
            "ones1_r": c_pool.tile([1, 128], F32, tag="ones1", name="ones1_r"),
            "ones_b": c_pool.tile([128, 1], BF16, tag="onesbf", name="ones_b"),
            "cos_t": c_pool.tile([128, T], F16, tag="cos", name="cos_t"),
            "sin_t": c_pool.tile([128, T], F16, tag="sin", name="sin_t"),
            "lam1c": c_pool.tile([128, 1], F32, tag="lam1", name="lam1c"),
        }
        nc.vector.memset(P["ones_r"][:], 1.0)
        nc.vector.memset(P["ones1_r"][:], 1.0)
        nc.vector.memset(P["ones_b"][:], 1.0)
        # rope tables: rows 0:32 real freqs, 32:64 freq=0 (cos=1, sin=0),
        # duplicated at partition offset 64 for the two rope halves.
        # csg rows (width 1024): 0:64 = cos [32, 2048], 64:128 = sin [32, 2048]
        for off in (0, 64):
            nc.sync.dma_start(P["cos_t"][off:off + 32, :], csg[0:64, :])
            nc.sync.dma_start(P["sin_t"][off:off + 32, :], csg[64:128, :])
            nc.vector.memset(P["cos_t"][off + 32:off + 64, :], 1.0)
            nc.vector.memset(P["sin_t"][off + 32:off + 64, :], 0.0)
        # broadcast lambda_1 (an fp16 scalar in the packed input) to [128,1]
        with ExitStack() as lctx:
            lam_pool = lctx.enter_context(tc.tile_pool(name="lamp", bufs=1))
            pl_pool = lctx.enter_context(tc.tile_pool(name="pl", bufs=1, space="PSUM"))
            lam_t = lam_pool.tile([1, 2], F16, tag="lam", name="lam_t")
            lam_f = lam_pool.tile([1, 2], R32, tag="lamf", name="lam_f")
            nc.sync.dma_start(lam_t[:], pk[LAM_OFF:LAM_OFF + 1, 0:2])
            nc.vector.tensor_copy(lam_f[:], lam_t[:])
            plam = pl_pool.tile([128, 2], F32, tag="plam")
            nc.tensor.matmul(plam[:], P["ones1_r"][:].bitcast(R32),
                             lam_f[:], start=True, stop=True)
            nc.scalar.copy(P["lam1c"][:], plam[:, 0:1])

        with ExitStack() as ctx_a:
            _phase_a(nc, tc, ctx_a, dram, P, xg, wqkg, wvg)
        with ExitStack() as ctx_b:
            _phase_b(nc, tc, ctx_b, P)
        with ExitStack() as ctx_c:
            _phase_c(nc, tc, ctx_c, P, cwg, opart)

        nc.gpsimd.collective_compute(
            "ReduceScatter", mybir.AluOpType.add, replica_groups=PAIRS,
            ins=[opart.opt()], outs=[oscat.opt()],
        )
        # pack the fp16 token-half output to 12 bits (round, split planes)
        with ExitStack() as octx:
            opk_pool = octx.enter_context(tc.tile_pool(name="opk", bufs=6))
            oh8 = dram["oh8"].ap()
            for t in range(8):
                ot16 = opk_pool.tile([128, DIM], F16, tag="ot")
                nc.sync.dma_start(ot16[:], oscat[t * 128:(t + 1) * 128, :])
                ur = opk_pool.tile([128, DIM], U16, tag="ur")
                nc.vector.tensor_scalar(ur[:], ot16[:].bitcast(U16), 8, None, ADD)
                hi16t = opk_pool.tile([128, DIM], U16, tag="ohi16")
                nc.vector.tensor_scalar(hi16t[:], ur[:], 8, None, SHR)
                hi8t = opk_pool.tile([128, DIM], U8, tag="ohi")
                nc.vector.tensor_copy(hi8t[:], hi16t[:])
                lo16t = opk_pool.tile([128, DIM], U16, tag="olo16")
                nc.vector.tensor_scalar(lo16t[:], ur[:], 4, 15, SHR, BAND)
                lo4t = opk_pool.tile([128, DIM], U8, tag="olo")
                nc.vector.tensor_copy(lo4t[:], lo16t[:])
                nibt = opk_pool.tile([128, DIM // 2], U8, tag="onib")
                nc.vector.tensor_scalar(nibt[:], lo4t[:, 0:DIM // 2], 4, None, SHL)
                nc.vector.tensor_tensor(nibt[:], nibt[:], lo4t[:, DIM // 2:DIM], BOR)
                nc.sync.dma_start(oh8[OHI_OFF + t * 64:OHI_OFF + (t + 1) * 64, :],
                                  hi8t[:])
                nc.sync.dma_start(oh8[ONIB_OFF + t * 32:ONIB_OFF + (t + 1) * 32, :],
                                  nibt[:])

    nc.compile()
    return nc


def _planes12(a32):
    """[B, T, DIM] f32 -> (hi [B,T,DIM] u8, lo4 [B,T,DIM] u8) of the fp16
    values rounded to 12 bits (round-half-up on the dropped 4 mantissa bits)."""
    u = a32.astype(np.float16).view(np.uint16) + np.uint16(8)
    hi = (u >> 8).astype(np.uint8)
    lo4 = ((u >> 4) & np.uint16(15)).astype(np.uint8)
    return hi, lo4


def _prep_inputs(x, ve, qkv_w, lambdas, c_proj_w):
    if "tables" not in _cache:
        _cache["tables"] = _rope_tables()
    cossin = _cache["tables"]                      # [64, T] fp16
    xhi, xlo = _planes12(np.asarray(x, np.float32))
    vhi, vlo = _planes12(np.asarray(ve, np.float32))
    qw, kw, vw = qkv_w[0], qkv_w[1], qkv_w[2]

    wqkT, wvT, cwT = [], [], []
    for g in range(HG):
        heads = range(g * HPG, (g + 1) * HPG)
        rows = np.concatenate(
            [np.concatenate([qw[h * D:(h + 1) * D], kw[h * D:(h + 1) * D]])
             for h in heads]
        )                                    # [1024, DIM]
        wqkT.append(np.ascontiguousarray(rows.T).astype(np.float16))
        vcols = slice(g * FV, (g + 1) * FV)
        wvT.append(np.ascontiguousarray(
            (lambdas[0] * vw[vcols]).T).astype(np.float16))
        cwT.append(np.ascontiguousarray(c_proj_w[:, vcols].T).astype(np.float16))

    lamrow = np.zeros((1, 1024), np.float16)
    lamrow[0, 0] = np.float16(lambdas[1])

    in_maps = []
    for core in range(B * HG):
        b, g = core // HG, core % HG
        cols = slice(g * XH, (g + 1) * XH)
        pk16 = np.concatenate([
            wqkT[g][b * 256:(b + 1) * 256],
            wvT[g][b * 256:(b + 1) * 256].reshape(-1, 1024),
            cwT[g][b * 128:(b + 1) * 128],
            cossin[core * 8:(core + 1) * 8].reshape(-1, 1024),
            lamrow,
        ], axis=0)
        xl = xlo[b][:, cols]
        vl = vlo[b][:, cols]
        pk8 = np.concatenate([
            xhi[b][:, cols].reshape(-1, 2048),
            ((xl[:, 0:256] << 4) | xl[:, 256:512]).reshape(-1, 2048),
            vhi[b][:, cols].reshape(-1, 2048),
            ((vl[:, 0:256] << 4) | vl[:, 256:512]).reshape(-1, 2048),
            pk16.view(np.uint8).reshape(-1, 2048),
        ], axis=0)
        in_maps.append({"pk8": pk8})
    return in_maps


def kernel(x, ve, qkv_w, lambdas, c_proj_w):
    x = np.asarray(x, np.float32)
    ve = np.asarray(ve, np.float32)
    qkv_w = np.asarray(qkv_w, np.float32).reshape(3, H * D, DIM)
    lambdas = np.asarray(lambdas, np.float32)
    c_proj_w = np.asarray(c_proj_w, np.float32)

    if "nc" not in _cache:
        _cache["nc"] = _build_program()
    nc = _cache["nc"]

    in_maps = _prep_inputs(x, ve, qkv_w, lambdas, c_proj_w)
    res = run_bass_kernel_spmd(nc, in_maps, list(range(B * HG))).results

    out = np.empty((B, T, DIM), np.float32)
    for b in range(B):
        for g in range(HG):
            o8 = res[HG * b + g]["oh8"]
            hi = o8[OHI_OFF:OHI_OFF + 512].reshape(T // 2, DIM)
            nib = o8[ONIB_OFF:ONIB_OFF + 256].reshape(T // 2, DIM // 2)
            u = hi.astype(np.uint16) << 8
            u[:, 0:DIM // 2] |= (nib >> 4).astype(np.uint16) << 4
            u[:, DIM // 2:] |= (nib & np.uint8(15)).astype(np.uint16) << 4
            out[b, g * (T // 2):(g + 1) * (T // 2)] = u.view(np.float16)
    return out
